# revision 1
# baseline (speedup 1.0000x reference)
"""Trainium2 Bass kernel for masked multi-head attention.

Reference computation (B=4, T=2048, D=1024, H=16, dh=64):
    qp = q @ Wq.T + bq ; kp = k @ Wk.T + bk ; vp = v @ Wv.T + bv
    s  = (qh @ khT) / 8 ; s = where(mask, -1e6, s) ; p = softmax(s)
    o  = p @ vh ; y = o @ Wo.T + bo

Sharding: 8 cores = (batch b in 0..3) x (head-group g in 0..1).
Each core handles batch b and 8 heads (512 channels), computes a partial
y^T (output projection over its 512 channels); host sums core pairs,
transposes, and adds the bias terms.

Per-core device algorithm (everything in transposed "T-major" layouts so
no on-device transposes are needed):
  A) qpT[c,t] = sum_m WqT[m,c] * qT[m,t]   (float32r matmuls, psum acc)
     kpT likewise.  1/8 score scale folded into WqT host-side.
  B) vp[t,c]  = sum_m vT[m,t] * WvT[m,c]   (untransposed; stored bf16 in
     a [t, 8*65] layout with a ones-column per head for row-sums)
  C) per (qcol, head): ST[k,q] = khT.T @ qhT (f32r) -> exp on ACT (bf16)
     -> multiply by maskT tile (DVE, bf16 2x) -> OT[d,q] (+= over k-tiles,
     PE, bf16) with row 64 = sum_k p~[k,q] (ones column).
     Normalize: otn = OT[0:64] * (1/r) broadcast (DVE + gpsimd bcast).
  D) yT[j,t] = sum_c wot[c,j] * otn[c,t]   (bf16) -> DMA out.
"""

import sys
import numpy as np

for _p in ("/opt/trn_rl_repo",):
    if _p not in sys.path:
        sys.path.insert(0, _p)

import ml_dtypes
from contextlib import ExitStack

import concourse.bass as bass
import concourse.tile as tile
from concourse import bacc, mybir
from concourse import bass_utils

B, T, D, H = 4, 2048, 1024, 16
DH = 64          # head dim
HC = 8           # heads per core
C = HC * DH      # 512 channels per core
F32 = mybir.dt.float32
F32R = mybir.dt.float32r
BF16 = mybir.dt.bfloat16
EXP = mybir.ActivationFunctionType.Exp

_CACHED = {}
OT_BUFS = 1
D_MODE = "interleaved"  # or "after"


def _build_nc():
    nc = bacc.Bacc("TRN2", target_bir_lowering=False, debug=False,
                   enable_asserts=False)
    qt = nc.dram_tensor("qt", [D, T], BF16, kind="ExternalInput").ap()
    kt = nc.dram_tensor("kt", [D, T], BF16, kind="ExternalInput").ap()
    vt = nc.dram_tensor("vt", [D, T], BF16, kind="ExternalInput").ap()
    wqt = nc.dram_tensor("wqt", [D, C], BF16, kind="ExternalInput").ap()
    wkt = nc.dram_tensor("wkt", [D, C], BF16, kind="ExternalInput").ap()
    wvt = nc.dram_tensor("wvt", [D, C], BF16, kind="ExternalInput").ap()
    wot = nc.dram_tensor("wot", [C, D], BF16, kind="ExternalInput").ap()
    maskt = nc.dram_tensor("maskt", [T, T], BF16, kind="ExternalInput").ap()
    bqt = nc.dram_tensor("bqt", [C, 1], F32, kind="ExternalInput").ap()
    bkt = nc.dram_tensor("bkt", [C, 1], F32, kind="ExternalInput").ap()
    yt = nc.dram_tensor("yt", [D, T], F32, kind="ExternalOutput").ap()

    with tile.TileContext(nc) as tc, ExitStack() as ctx:
        _emit(ctx, tc, qt, kt, vt, wqt, wkt, wvt, wot, maskt, bqt, bkt, yt)
    nc.compile()
    return nc



def _emit_d_jbs(nc, pypool, ye, wo_all, otn_sb, qc, jbs, altpool=None):
    NCT = C // 128
    for jb in jbs:
        pool = altpool if (altpool is not None and jb % 2) else pypool
        tg = "pv" if (altpool is not None and jb % 2) else "py"
        ps = pool.tile([128, 512], F32, tag=tg, name="psy")
        for ct in range(NCT):
            lhs = wo_all[:, ct * D + jb * 128:ct * D + (jb + 1) * 128]
            rhs = otn_sb[ct][:, qc * 512:(qc + 1) * 512]
            nc.tensor.matmul(ps[:, :], lhs, rhs,
                             start=(ct == 0), stop=(ct == NCT - 1))
        nc.vector.tensor_copy(ye[:, (jb % 2) * 512:(jb % 2 + 1) * 512],
                              ps[:, :])


def _emit_d_store(nc, ye, yt, qc, pair):
    nc.sync.dma_start(
        yt.rearrange("(jb p) t -> p jb t",
                     p=128)[:, pair * 2:(pair + 1) * 2,
                            qc * 512:(qc + 1) * 512],
        ye.rearrange("p (jb t) -> p jb t", jb=2))


def _emit(ctx, tc, qt, kt, vt, wqt, wkt, wvt, wot, maskt, bqt, bkt, yt):
    nc = tc.nc
    NKT = T // 128      # 16 k-tiles
    NQC = 4             # q columns of 512
    NCT = C // 128      # 4 channel tiles (= head pairs)
    NM = D // 128       # 8 contraction tiles

    # ---- persistent SBUF arrays -------------------------------------
    persist = ctx.enter_context(tc.tile_pool(name="persist", bufs=1))
    qpt_sb = [persist.tile([128, T], F32R, tag=f"qpt{i}", name=f"qpt{i}")
              for i in range(NCT)]
    kpt_sb = [persist.tile([128, T], F32R, tag=f"kpt{i}", name=f"kpt{i}")
              for i in range(NCT)]
    VPW = HC * (DH + 1)  # 520
    vp_ext = persist.tile([128, NKT * VPW], BF16, tag="vpext", name="vpext")
    otn_sb = [persist.tile([128, T], BF16, tag=f"otn{i}", name=f"otn{i}")
              for i in range(NCT)]
    bias_sb = persist.tile([128, 2 * NCT], F32, tag="bias", name="bias")
    wo_all = persist.tile([128, NCT * D], BF16, tag="wo", name="wo_all")

    nc.gpsimd.memset(vp_ext[:, :], 1.0)
    nc.sync.dma_start(bias_sb[:, 0:NCT],
                      bqt.rearrange("(c p) o -> p (c o)", p=128))
    nc.sync.dma_start(bias_sb[:, NCT:2 * NCT],
                      bkt.rearrange("(c p) o -> p (c o)", p=128))

    # mask tiles double-buffered; prefetch qc0/qc1 before phase A so
    # attention's first q-column never waits on its mask.
    mpool = ctx.enter_context(tc.tile_pool(name="mask", bufs=2))
    mask_tiles = {}
    msrc = maskt.rearrange("(kt p) q -> p kt q", p=128)
    def _load_mask(qc):
        m_all = mpool.tile([128, NKT * 512], BF16, tag="m", name="m_all")
        nc.sync.dma_start(m_all.rearrange("p (kt q) -> p kt q", kt=NKT),
                          msrc[:, :, qc * 512:(qc + 1) * 512])
        mask_tiles[qc] = m_all

    # PSUM left stack: pproj(8) -> pv(4) -> st(4)+ot(2); right: py(2).
    ppool = tc.alloc_tile_pool(name="pproj", bufs=2, space="PSUM")

    # ---- Phase A: K projection + first quarter of Q up front --------
    # Q quarters tq=1..3 are deferred into the attention pipeline.
    wpoolq = ctx.enter_context(tc.tile_pool(name="wtsq", bufs=1))
    wq_all = wpoolq.tile([128, NM * C], BF16, tag="wq", name="wq_all")
    with tc.tile_pool(name="wts", bufs=1) as wpool, \
         tc.tile_pool(name="xin", bufs=4) as xpool:
        wk_all = wpool.tile([128, NM * C], BF16, tag="w", name="wk_all")
        for wdst_t, wdram in ((wk_all, wkt), (wq_all, wqt)):
            wsrc = wdram.rearrange("(m p) c -> p m c", p=128)
            wdst = wdst_t.rearrange("p (m c) -> p m c", m=NM)
            nc.sync.dma_start(wdst[:, 0:1], wsrc[:, 0:1])
            nc.sync.dma_start(wdst[:, 1:NM], wsrc[:, 1:NM])
        for pi, (w_all, xdram, dst, boff, ths) in enumerate(
                ((wk_all, kt, kpt_sb, NCT, (0, 1)),
                 (wq_all, qt, qpt_sb, 0, (0,)))):
            for th in ths:               # halves of T
                tcs = (0, 1) if pi == 0 else (0,)
                psq = {}
                for m in range(NM):
                    xm = xpool.tile([128, 1024], BF16, tag="x", name="x")
                    nc.sync.dma_start(
                        xm[:, :], xdram[m * 128:(m + 1) * 128,
                                        th * 1024:(th + 1) * 1024])
                    for ct in range(NCT):
                        lhs = w_all[:, m * C + ct * 128:m * C + (ct + 1) * 128]
                        for tc2 in tcs:
                            if m == 0:
                                psq[(ct, tc2)] = ppool.tile(
                                    [128, 512], F32, tag=f"pp{ct}",
                                    name=f"pp{ct}")
                            nc.tensor.matmul(
                                psq[(ct, tc2)][:, :],
                                lhs, xm[:, tc2 * 512:(tc2 + 1) * 512],
                                start=(m == 0), stop=(m == NM - 1))
                for ct in range(NCT):
                    for tc2 in tcs:
                        tq = th * 2 + tc2
                        nc.vector.tensor_scalar_add(
                            dst[ct][:, tq * 512:(tq + 1) * 512],
                            psq[(ct, tc2)][:, :],
                            bias_sb[:, boff + ct:boff + ct + 1])
    ppool.release()
    _load_mask(0)
    nc.sync.dma_start(wo_all.rearrange("p (c j) -> p c j", c=NCT),
                      wot.rearrange("(c p) j -> p c j", p=128))

    # ---- Phase B: V projection (emitted interleaved into phase C) ---
    pvpool = tc.alloc_tile_pool(name="pv", bufs=1, space="PSUM")
    wpool2 = ctx.enter_context(tc.tile_pool(name="wts2", bufs=1))
    vtpool = ctx.enter_context(tc.tile_pool(name="vtin", bufs=8))
    wv_all = wpool2.tile([128, NM * C], BF16, tag="wv", name="wv")
    nc.sync.dma_start(wv_all.rearrange("p (m c) -> p m c", m=NM),
                      wvt.rearrange("(m p) c -> p m c", p=128))
    vtm_tiles = {}
    xq_tiles = {}

    def emit_q_tail(j):
        # j in 0..11 -> (tq, ct); projects qpt[:, tq-quarter] for tile ct
        tq, ct = 1 + j // NCT, j % NCT
        if ct == 0:
            for m in range(NM):
                xq = vtpool.tile([128, 512], BF16, tag="vt",
                                 name=f"xq{m}")
                nc.sync.dma_start(
                    xq[:, :], qt[m * 128:(m + 1) * 128,
                                 tq * 512:(tq + 1) * 512])
                xq_tiles[m] = xq
        ps = pvpool.tile([128, C], F32, tag="pv", name="pvq")
        for m in range(NM):
            lhs = wq_all[:, m * C + ct * 128:m * C + (ct + 1) * 128]
            nc.tensor.matmul(ps[:, 0:512], lhs, xq_tiles[m][:, :],
                             start=(m == 0), stop=(m == NM - 1))
        nc.vector.tensor_scalar_add(
            qpt_sb[ct][:, tq * 512:(tq + 1) * 512], ps[:, 0:512],
            bias_sb[:, ct:ct + 1])

    def emit_b(tt):
        tg, i = divmod(tt, 4)
        if i == 0:
            for m in range(NM):
                vtm = vtpool.tile([128, 512], BF16, tag="vt",
                                  name=f"vtm{m}")
                nc.sync.dma_start(
                    vtm[:, :], vt[m * 128:(m + 1) * 128,
                                  tg * 512:(tg + 1) * 512])
                vtm_tiles[m] = vtm
        ps = pvpool.tile([128, C], F32, tag="pv", name="pv")
        for m in range(NM):
            nc.tensor.matmul(ps[:, :],
                             vtm_tiles[m][:, i * 128:(i + 1) * 128],
                             wv_all[:, m * C:(m + 1) * C],
                             start=(m == 0), stop=(m == NM - 1))
        dstv = vp_ext[:, tt * VPW:(tt + 1) * VPW].rearrange(
            "p (h e) -> p h e", h=HC)[:, :, 0:DH]
        nc.vector.tensor_copy(
            dstv, ps.rearrange("p (h e) -> p h e", h=HC))

    # ---- Phase C: attention, with phase-D block interleaved per qc --
    stpool = tc.alloc_tile_pool(name="st", bufs=2, space="PSUM", side="right")
    pypool = tc.alloc_tile_pool(name="py", bufs=1, space="PSUM", side="right")
    otpool = tc.alloc_tile_pool(name="ot", bufs=OT_BUFS, space="PSUM", side="right")
    with tc.tile_pool(name="pt", bufs=4) as ptpool, \
         tc.tile_pool(name="ptm", bufs=4) as ptmpool, \
         tc.tile_pool(name="nrm", bufs=2) as nrmpool, \
         tc.tile_pool(name="yev", bufs=1) as ypool:
        NG = NQC * NCT * NKT          # 256 pipeline steps
        ptms = {}                     # g -> masked-prob tile
        ots = {}                      # (qc, hp) -> [ot_h0, ot_h1]
        ye_box = [None]

        def _coords(g):
            qc, r = divmod(g, NCT * NKT)
            hp, ktile = divmod(r, NKT)
            return qc, hp, ktile

        def s1(g):
            # QK^T -> exp -> mask multiply for step g (2 steps ahead of PV)
            qc, hp, ktile = _coords(g)
            if hp == 0 and ktile == 0 and qc + 1 < NQC \
                    and qc + 1 not in mask_tiles:
                _load_mask(qc + 1)
            m_all = mask_tiles[qc]
            st = stpool.tile([128, 1024], F32, tag="st", name="st")
            for h in range(2):
                lhs = kpt_sb[hp][h * 64:(h + 1) * 64,
                                 ktile * 128:(ktile + 1) * 128]
                rhs = qpt_sb[hp][h * 64:(h + 1) * 64,
                                 qc * 512:(qc + 1) * 512]
                nc.tensor.matmul(st[:, h * 512:(h + 1) * 512],
                                 lhs, rhs, start=True, stop=True)
            pt = ptpool.tile([128, 1024], BF16, tag="pt", name="pt")
            nc.scalar.activation(pt[:, :], st[:, :], EXP)
            ptm = ptmpool.tile([128, 1024], BF16, tag="ptm", name="ptm")
            msl = m_all[:, ktile * 512:(ktile + 1) * 512]
            for h in range(2):
                nc.vector.tensor_tensor(
                    ptm[:, h * 512:(h + 1) * 512],
                    pt[:, h * 512:(h + 1) * 512], msl,
                    mybir.AluOpType.mult)
            ptms[g] = ptm

        def s2(g):
            # PV accumulation for step g
            qc, hp, ktile = _coords(g)
            if ktile == 0:
                ots[(qc, hp)] = [otpool.tile([65, 512], F32, tag=f"ot{h}",
                                             name=f"ot{h}")
                                 for h in range(2)]
            ot2 = ots[(qc, hp)]
            ptm = ptms.pop(g)
            for h in range(2):
                hg = hp * 2 + h
                vsl = vp_ext[:, ktile * VPW + hg * 65:
                             ktile * VPW + (hg + 1) * 65]
                nc.tensor.matmul(ot2[h][:, :], vsl,
                                 ptm[:, h * 512:(h + 1) * 512],
                                 start=(ktile == 0),
                                 stop=(ktile == NKT - 1))

        LEAD = 4
        for g0 in range(LEAD):
            s1(g0)
            if g0 < NKT:
                emit_b(g0)
        qtail_sched = {}
        for j in range(12):
            tq = 1 + j // NCT
            qtail_sched[18 + (tq - 1) * 56 + (j % NCT) * 8] = j
        for g in range(NG):
            s2(g)
            if g + LEAD < NG:
                s1(g + LEAD)
                if g + LEAD < NKT:
                    emit_b(g + LEAD)
            if g in qtail_sched:
                emit_q_tail(qtail_sched[g])
            qc, hp, ktile = _coords(g)
            if ktile != NKT - 1:
                continue
            # head-pair epilogue: normalize + evacuate to otn
            ot2 = ots.pop((qc, hp))
            for h in range(2):
                recip = nrmpool.tile([1, 512], F32, tag="rc", name="recip")
                rep = nrmpool.tile([64, 512], F32, tag="rep", name="rep")
                nc.vector.reciprocal(recip[:, :], ot2[h][64:65, :])
                nc.gpsimd.partition_broadcast(rep[:, :], recip[:, :])
                nc.vector.tensor_tensor(
                    otn_sb[hp][h * 64:(h + 1) * 64,
                               qc * 512:(qc + 1) * 512],
                    ot2[h][0:64, :], rep[:, :], mybir.AluOpType.mult)
            # spread previous qcol's output projection across this qcol
            if D_MODE == "interleaved" and qc > 0:
                ye_box[0] = ypool.tile([128, 2 * 512], F32,
                                       tag="ye", name="ye")
                _emit_d_jbs(nc, pypool, ye_box[0], wo_all, otn_sb,
                            qc - 1, range(hp * 2, hp * 2 + 2))
                _emit_d_store(nc, ye_box[0], yt, qc - 1, hp)
            if D_MODE == "interleaved" and qc == NQC - 1 and hp == NCT - 1:
                for pair in range(4):
                    ye = ypool.tile([128, 2 * 512], F32, tag="ye", name="ye")
                    _emit_d_jbs(nc, pypool, ye, wo_all, otn_sb, qc,
                                range(pair * 2, pair * 2 + 2),
                                altpool=pvpool)
                    _emit_d_store(nc, ye, yt, qc, pair)
    otpool.release()
    pypool.release()
    stpool.release()
    pvpool.release()


def kernel(q, k, v, mask, Wq, bq, Wk, bk, Wv, bv, Wo, bo, _trace=False):
    if "nc" not in _CACHED:
        _CACHED["nc"] = _build_nc()
    nc = _CACHED["nc"]

    q = np.asarray(q, np.float32)
    k = np.asarray(k, np.float32)
    v = np.asarray(v, np.float32)
    Wq = np.asarray(Wq, np.float32)
    Wk = np.asarray(Wk, np.float32)
    Wv = np.asarray(Wv, np.float32)
    Wo = np.asarray(Wo, np.float32)
    mask = np.asarray(mask)

    in_maps = []
    for core in range(8):
        b, g = divmod(core, 2)
        csl = slice(g * C, (g + 1) * C)
        im = {
            "qt": np.ascontiguousarray(q[b].T).astype(ml_dtypes.bfloat16),
            "kt": np.ascontiguousarray(k[b].T).astype(ml_dtypes.bfloat16),
            "vt": np.ascontiguousarray(v[b].T).astype(ml_dtypes.bfloat16),
            "wqt": np.ascontiguousarray((Wq[csl, :] / 8.0).T).astype(ml_dtypes.bfloat16),
            "wkt": np.ascontiguousarray(Wk[csl, :].T).astype(ml_dtypes.bfloat16),
            "wvt": np.ascontiguousarray(Wv[csl, :].T).astype(ml_dtypes.bfloat16),
            "wot": np.ascontiguousarray(Wo[:, csl].T).astype(
                ml_dtypes.bfloat16),
            "maskt": np.ascontiguousarray(
                (~mask[b, 0]).T.astype(np.float32)).astype(ml_dtypes.bfloat16),
            "bqt": np.ascontiguousarray(
                (np.asarray(bq, np.float32)[csl] / 8.0).reshape(C, 1)),
            "bkt": np.ascontiguousarray(
                np.asarray(bk, np.float32)[csl].reshape(C, 1)),
        }
        in_maps.append(im)

    res = bass_utils.run_bass_kernel_spmd(
        nc, in_maps, core_ids=list(range(8)), trace=_trace)
    if _trace:
        _CACHED["last_results"] = res
    outs = [r["yt"] for r in res.results]

    y = np.empty((B, T, D), np.float32)
    const = (Wo @ np.asarray(bv, np.float32)
             + np.asarray(bo, np.float32)).astype(np.float32)
    for b in range(B):
        y[b] = (outs[2 * b] + outs[2 * b + 1]).T + const
    return y



# revision 58
# speedup vs baseline: 1.2038x; 1.2038x over previous
"""Trainium2 Bass kernel for masked multi-head attention.

Reference computation (B=4, T=2048, D=1024, H=16, dh=64):
    qp = q @ Wq.T + bq ; kp = k @ Wk.T + bk ; vp = v @ Wv.T + bv
    s  = (qh @ khT) / 8 ; s = where(mask, -1e6, s) ; p = softmax(s)
    o  = p @ vh ; y = o @ Wo.T + bo

Sharding: 8 cores = (batch b in 0..3) x (head-group g in 0..1).
Each core handles batch b and 8 heads (512 channels), computes a partial
y^T (output projection over its 512 channels); host sums core pairs,
transposes, and adds the bias terms.

Per-core device algorithm (everything in transposed "T-major" layouts so
no on-device transposes are needed):
  A) qpT[c,t] = sum_m WqT[m,c] * qT[m,t]   (float32r matmuls, psum acc)
     kpT likewise.  1/8 score scale folded into WqT host-side.
  B) vp[t,c]  = sum_m vT[m,t] * WvT[m,c]   (untransposed; stored bf16 in
     a [t, 8*65] layout with a ones-column per head for row-sums)
  C) per (qcol, head): ST[k,q] = khT.T @ qhT (f32r) -> exp on ACT (bf16)
     -> multiply by maskT tile (DVE or GPSIMD, broadcast AP over both
     heads) -> transposed PV: OT[q,65] += ptm[k,q].T @ vp[k,65] per
     (head, q-subtile) with col 64 = row sums (ones column).
     Epilogue: recip rowsums (DVE), broadcast-multiply normalize (DVE),
     PE transpose via identity matmul -> [c,q] psum, DVE copy to otn.
  D) yT[j,t] = sum_c wot[c,j] * otn[c,t]   (bf16) -> DMA out.
"""

import sys
import numpy as np

for _p in ("/opt/trn_rl_repo",):
    if _p not in sys.path:
        sys.path.insert(0, _p)

import ml_dtypes
from contextlib import ExitStack

import concourse.bass as bass
import concourse.tile as tile
from concourse import bacc, mybir
from concourse import bass_utils

B, T, D, H = 4, 2048, 1024, 16
DH = 64          # head dim
HC = 8           # heads per core
C = HC * DH      # 512 channels per core
F32 = mybir.dt.float32
F32R = mybir.dt.float32r
BF16 = mybir.dt.bfloat16
EXP = mybir.ActivationFunctionType.Exp

_CACHED = {}
GP_MASK_MOD = 10 ** 9   # g % GP_MASK_MOD == 1 -> mask multiply on gpsimd


def _build_nc():
    nc = bacc.Bacc("TRN2", target_bir_lowering=False, debug=False,
                   enable_asserts=False)
    qt = nc.dram_tensor("qt", [D, T], BF16, kind="ExternalInput").ap()
    kt = nc.dram_tensor("kt", [D, T], BF16, kind="ExternalInput").ap()
    vt = nc.dram_tensor("vt", [D, T], BF16, kind="ExternalInput").ap()
    wqt = nc.dram_tensor("wqt", [D, C], BF16, kind="ExternalInput").ap()
    wkt = nc.dram_tensor("wkt", [D, C], BF16, kind="ExternalInput").ap()
    wvt = nc.dram_tensor("wvt", [D, C], BF16, kind="ExternalInput").ap()
    wot = nc.dram_tensor("wot", [C, D], BF16, kind="ExternalInput").ap()
    maskt = nc.dram_tensor("maskt", [T, T], BF16, kind="ExternalInput").ap()
    bqt = nc.dram_tensor("bqt", [C, 1], F32, kind="ExternalInput").ap()
    bkt = nc.dram_tensor("bkt", [C, 1], F32, kind="ExternalInput").ap()
    identt = nc.dram_tensor("identt", [128, 128], BF16,
                            kind="ExternalInput").ap()
    yt = nc.dram_tensor("yt", [D, T], F32, kind="ExternalOutput").ap()

    with tile.TileContext(nc) as tc, ExitStack() as ctx:
        _emit(ctx, tc, qt, kt, vt, wqt, wkt, wvt, wot, maskt, bqt, bkt,
              identt, yt)
    nc.compile()
    return nc


def _bcast(ap, reps, inner):
    """Repeat `ap`'s last `inner` elements `reps` times: [p, reps, inner]
    with a 0-stride outer dim."""
    return bass.AP(ap.tensor, ap.offset, [ap.ap[0], [0, reps], [1, inner]])


def _bcast_inner(ap, outer, reps):
    """Broadcast each of `ap`'s `outer` elements `reps` times:
    [p, outer, reps] with a 0-stride inner dim."""
    return bass.AP(ap.tensor, ap.offset, [ap.ap[0], [1, outer], [0, reps]])


def _emit_d_jbs(nc, pypool, ye, wo_all, otn_sb, qc, jbs, tag="py"):
    NCT = C // 128
    for jb in jbs:
        ps = pypool.tile([128, 512], F32, tag=tag, name="psy")
        for ct in range(NCT):
            lhs = wo_all[:, ct * D + jb * 128:ct * D + (jb + 1) * 128]
            rhs = otn_sb[ct][:, qc * 512:(qc + 1) * 512]
            nc.tensor.matmul(ps[:, :], lhs, rhs,
                             start=(ct == 0), stop=(ct == NCT - 1))
        nc.vector.tensor_copy(ye[:, (jb % 2) * 512:(jb % 2 + 1) * 512],
                              ps[:, :])


def _emit_d_store(nc, ye, yt, qc, pair):
    # gpsimd (swdge) queue: store DMAs wait on DVE ye-copies and must not
    # block input loads queued behind them on the SP queue.
    nc.gpsimd.dma_start(
        yt.rearrange("(jb p) t -> p jb t",
                     p=128)[:, pair * 2:(pair + 1) * 2,
                            qc * 512:(qc + 1) * 512],
        ye.rearrange("p (jb t) -> p jb t", jb=2))


def _emit(ctx, tc, qt, kt, vt, wqt, wkt, wvt, wot, maskt, bqt, bkt, identt,
          yt):
    nc = tc.nc
    NKT = T // 128      # 16 k-tiles
    NQC = 4             # q columns of 512
    NCT = C // 128      # 4 channel tiles (= head pairs)
    NM = D // 128       # 8 contraction tiles

    # ---- persistent SBUF arrays -------------------------------------
    persist = ctx.enter_context(tc.tile_pool(name="persist", bufs=1))
    qpt_sb = [persist.tile([128, T], BF16, tag=f"qpt{i}", name=f"qpt{i}")
              for i in range(NCT)]
    kpt_sb = [persist.tile([128, T], BF16, tag=f"kpt{i}", name=f"kpt{i}")
              for i in range(NCT)]
    VPW = HC * (DH + 1)  # 520
    vp_ext = persist.tile([128, NKT * VPW], BF16, tag="vpext", name="vpext")
    otn_sb = [persist.tile([128, T], BF16, tag=f"otn{i}", name=f"otn{i}")
              for i in range(NCT)]
    bias_sb = persist.tile([128, 2 * NCT], F32, tag="bias", name="bias")
    wo_all = persist.tile([128, NCT * D], BF16, tag="wo", name="wo_all")
    ident_sb = persist.tile([128, 128], BF16, tag="ident", name="ident_sb")

    # mask tiles double-buffered; qc0 mask first so attention never waits.
    mpool = ctx.enter_context(tc.tile_pool(name="mask", bufs=2))
    mask_tiles = {}
    msrc = maskt.rearrange("(kt p) q -> p kt q", p=128)

    def _load_mask(qc):
        # Later masks use the ACT dge queue: a mask load WAR-waits on DVE
        # mults of the retiring tile and must not block input loads behind
        # it on the SP queue.  qc0 (no WAR) stays on SP for strict order.
        m_all = mpool.tile([128, NKT * 512], BF16, tag="m", name="m_all")
        eng = nc.sync if qc == 0 else nc.gpsimd
        eng.dma_start(m_all.rearrange("p (kt q) -> p kt q", kt=NKT),
                      msrc[:, :, qc * 512:(qc + 1) * 512])
        mask_tiles[qc] = m_all

    nc.gpsimd.memset(vp_ext[:, :], 1.0)

    # PSUM: pproj(4, released after A) + py(1) | st(4) + ot(2) + trans(1).
    pypool = tc.alloc_tile_pool(name="py", bufs=1, space="PSUM",
                                side="right")
    ppool = tc.alloc_tile_pool(name="pproj", bufs=1, space="PSUM")

    wpoolq = ctx.enter_context(tc.tile_pool(name="wtsq", bufs=1))
    wq_all = wpoolq.tile([128, NM * C], BF16, tag="wq", name="wq_all")
    wpool2 = ctx.enter_context(tc.tile_pool(name="wts2", bufs=1))
    wv_all = wpool2.tile([128, NM * C], BF16, tag="wv", name="wv")
    vtpool = ctx.enter_context(tc.tile_pool(name="vtin", bufs=8))
    vtm_tiles = {}   # (tg, hp) -> [8 input tiles]
    xq_tiles = {}    # tq -> [8 input tiles]

    qsrc = qt.rearrange("(m p) t -> p m t", p=128)
    vsrc = vt.rearrange("(m p) t -> p m t", p=128)

    def _load_xq(tq):
        if tq in xq_tiles:
            return
        xqb = vtpool.tile([128, NM, 512], BF16, tag="xq", bufs=1,
                          name="xqb")
        nc.sync.dma_start(xqb[:, :, :],
                          qsrc[:, :, tq * 512:(tq + 1) * 512])
        xq_tiles.clear()
        xq_tiles[tq] = xqb

    def emit_q_chunk(tq, ct):
        # projects qpt[:, tq-quarter] for head-pair ct (8 matmuls, 1 bank)
        _load_xq(tq)
        xqb = xq_tiles[tq]
        ps = pypool.tile([128, 512], F32, tag="py", name="pvq")
        for m in range(NM):
            lhs = wq_all[:, m * C + ct * 128:m * C + (ct + 1) * 128]
            nc.tensor.matmul(ps[:, :], lhs, xqb[:, m, :],
                             start=(m == 0), stop=(m == NM - 1))
        nc.vector.tensor_scalar_add(
            qpt_sb[ct][:, tq * 512:(tq + 1) * 512], ps[:, :],
            bias_sb[:, ct:ct + 1])

    def _load_vtm(tg, hp=0):
        # cached per t-group (same data for every head-pair)
        if tg in vtm_tiles or not 0 <= tg < 4:
            return
        vtb = vtpool.tile([128, NM, 512], BF16, tag="vt", bufs=4,
                          name="vtb")
        nc.sync.dma_start(vtb[:, :, :],
                          vsrc[:, :, tg * 512:(tg + 1) * 512])
        vtm_tiles[tg] = vtb

    def emit_v(kt, hp):
        # V projection pair: head-pair hp, t-blocks kt and kt+1 (16
        # matmuls N=128 + one copy -> one psum-bank cycle per 2 steps)
        tg = kt // 4
        _load_vtm(tg)
        if kt % 4 == 0:     # prefetch the next t-group's inputs
            _load_vtm(tg + 1)
        vtb = vtm_tiles[tg]
        ps = pypool.tile([128, 512], F32, tag="py", name="pv")
        for j in range(2):
            i = kt % 4 + j
            for m in range(NM):
                nc.tensor.matmul(
                    ps[:, j * 128:(j + 1) * 128],
                    vtb[:, m, i * 128:(i + 1) * 128],
                    wv_all[:, m * C + hp * 128:m * C + (hp + 1) * 128],
                    start=(j == 0 and m == 0),
                    stop=(j == 1 and m == NM - 1))
        pstr = ps.ap[0][0]
        vstr = vp_ext.ap[0][0]
        dstv = bass.AP(vp_ext.tensor,
                       vp_ext.offset + kt * VPW + hp * 130,
                       [[vstr, 128], [VPW, 2], [65, 2], [1, DH]])
        srcv = bass.AP(ps.tensor, ps.offset,
                       [[pstr, 128], [128, 2], [64, 2], [1, DH]])
        nc.vector.tensor_copy(dstv, srcv)

    # ---- Phase A: K projection; q0/hp0 comes via emit_q_chunk -------
    warm = persist.tile([1, 2], F32, tag="warm", name="warm")
    with tc.tile_pool(name="wts", bufs=1) as wpool, \
         tc.tile_pool(name="xin", bufs=4) as xpool:
        wk_all = wpool.tile([128, NM * C], BF16, tag="w", name="wk_all")
        ksrc = kt.rearrange("(m p) t -> p m t", p=128)
        wksrc = wkt.rearrange("(m p) c -> p m c", p=128)
        wkv = wk_all.rearrange("p (m c) -> p m c", m=NM)
        # wk half + first k inputs first so matmuls start ~4us in
        nc.sync.dma_start(wkv[:, 0:4], wksrc[:, 0:4])
        xall = {}
        for th in (0, 1):
            for mh in range(2):
                xmb = xpool.tile([128, NM // 2, 1024], BF16, tag="x",
                                 name="x")
                nc.sync.dma_start(
                    xmb[:, :, :],
                    ksrc[:, mh * 4:(mh + 1) * 4,
                         th * 1024:(th + 1) * 1024])
                xall[(th, mh)] = xmb
                if th == 0 and mh == 0:
                    nc.sync.dma_start(wkv[:, 4:NM], wksrc[:, 4:NM])
                    nc.sync.dma_start(
                        bias_sb[:, NCT:2 * NCT],
                        bkt.rearrange("(c p) o -> p (c o)", p=128))
                    nc.sync.dma_start(
                        bias_sb[:, 0:NCT],
                        bqt.rearrange("(c p) o -> p (c o)", p=128))
                    nc.sync.dma_start(ident_sb[:, :], identt)
        # warm the ACT exp table while DMAs stream
        nc.gpsimd.memset(warm[:, :], 0.0)
        nc.scalar.activation(warm[:, :], warm[:, :], EXP)
        # then the attention-start loads: q weights, q0 inputs, wv
        nc.sync.dma_start(wq_all.rearrange("p (m c) -> p m c", m=NM),
                          wqt.rearrange("(m p) c -> p m c", p=128))
        _load_xq(0)
        nc.sync.dma_start(wv_all.rearrange("p (m c) -> p m c", m=NM),
                          wvt.rearrange("(m p) c -> p m c", p=128))
        for th in (0, 1):               # halves of T
            xh = [xall[(th, 0)], xall[(th, 1)]]
            for tc2 in (0, 1):
                psq = {}
                npass = th * 2 + tc2
                for m in range(NM):
                    for ct in range(NCT):
                        lhs = wk_all[:, m * C + ct * 128:
                                     m * C + (ct + 1) * 128]
                        if m == 0:
                            # rotate over 7 banks so pass N+1 never
                            # WAR-waits on pass N's bias-add
                            tg7 = (npass * 4 + ct) % 7
                            psq[ct] = ppool.tile(
                                [128, 512], F32, tag=f"pp{tg7}",
                                name=f"pp{tg7}")
                        nc.tensor.matmul(
                            psq[ct][:, :],
                            lhs,
                            xh[m // 4][:, m % 4,
                                       tc2 * 512:(tc2 + 1) * 512],
                            start=(m == 0), stop=(m == NM - 1))
                for ct in range(NCT):
                    tq = th * 2 + tc2
                    nc.vector.tensor_scalar_add(
                        kpt_sb[ct][:, tq * 512:(tq + 1) * 512],
                        psq[ct][:, :],
                        bias_sb[:, NCT + ct:NCT + ct + 1])
                if th == 1 and tc2 == 0:
                    # q0/hp0 here: its bias-add lands while DVE is free,
                    # so the warmup scores are not gated by K's adds.
                    emit_q_chunk(0, 0)
    ppool.release()

    # ---- Phase C: attention, with phase-D block interleaved per qc --
    stpool = tc.alloc_tile_pool(name="st", bufs=2, space="PSUM",
                                side="right")
    otpool = tc.alloc_tile_pool(name="ot", bufs=1, space="PSUM",
                                side="right")
    trpool = tc.alloc_tile_pool(name="tr", bufs=1, space="PSUM",
                                side="right")
    with tc.tile_pool(name="pt", bufs=5) as ptpool, \
         tc.tile_pool(name="ptm", bufs=4) as ptmpool, \
         tc.tile_pool(name="nrm", bufs=2) as nrmpool, \
         tc.tile_pool(name="oq", bufs=2) as oqpool, \
         tc.tile_pool(name="yev", bufs=2) as ypool:
        NG = NQC * NCT * NKT          # 256 pipeline steps
        ptms = {}                     # g -> masked-prob tile
        ots = {}                      # (qc, hp) -> [ot_h0, ot_h1]
        dstate = {}                   # rolling phase-D psum/ye tiles

        def _coords(g):
            qc, r = divmod(g, NCT * NKT)
            hp, ktile = divmod(r, NKT)
            return qc, hp, ktile

        def s1(g):
            # QK^T -> exp -> mask multiply for step g (LEAD ahead of PV)
            qc, hp, ktile = _coords(g)
            if hp == 1 and ktile == 0 and qc + 1 < NQC \
                    and qc + 1 not in mask_tiles:
                _load_mask(qc + 1)
            m_all = mask_tiles[qc]
            st = stpool.tile([128, 1024], F32, tag="st", name="st")
            for h in range(2):
                lhs = kpt_sb[hp][h * 64:(h + 1) * 64,
                                 ktile * 128:(ktile + 1) * 128]
                rhs = qpt_sb[hp][h * 64:(h + 1) * 64,
                                 qc * 512:(qc + 1) * 512]
                nc.tensor.matmul(st[:, h * 512:(h + 1) * 512],
                                 lhs, rhs, start=True, stop=True)
            pt = ptpool.tile([128, 1024], BF16, tag="pt", name="pt")
            nc.scalar.activation(pt[:, :], st[:, :], EXP)
            ptm = ptmpool.tile([128, 1024], BF16, tag="ptm", name="ptm")
            msl = m_all[:, ktile * 512:(ktile + 1) * 512]
            eng = nc.gpsimd if (g % GP_MASK_MOD == 1) else nc.vector
            eng.tensor_tensor(
                ptm.rearrange("p (t q) -> p t q", t=2),
                pt.rearrange("p (t q) -> p t q", t=2),
                _bcast(msl, 2, 512), mybir.AluOpType.mult)
            ptms[g] = ptm

        def s2(g):
            # transposed PV accumulation for step g
            qc, hp, ktile = _coords(g)
            if ktile == 0:
                ots[(qc, hp)] = [otpool.tile([128, 260], F32, tag=f"ot{h}",
                                             name=f"ot{h}")
                                 for h in range(2)]
            ot2 = ots[(qc, hp)]
            ptm = ptms.pop(g)
            for h in range(2):
                hg = hp * 2 + h
                vsl = vp_ext[:, ktile * VPW + hg * 65:
                             ktile * VPW + (hg + 1) * 65]
                for qs in range(4):
                    # one psum group per bank: the first matmul's start
                    # marks the whole zero region pending-zero, later
                    # slices replace-then-accumulate (has_written bits)
                    nc.tensor.matmul(
                        ot2[h][:, qs * 65:(qs + 1) * 65],
                        ptm[:, h * 512 + qs * 128:h * 512 + (qs + 1) * 128],
                        vsl,
                        start=(ktile == 0 and qs == 0),
                        stop=(ktile == NKT - 1 and qs == 3))

        otqs = {}                     # (qc, hp) -> [otq_h0, otq_h1]

        def epi_norm(qc, hp):
            # DVE-only: gather row sums, reciprocal, broadcast-multiply
            ot2 = ots.pop((qc, hp))
            pair = []
            for h in range(2):
                otv = ot2[h].rearrange("p (qs e) -> p qs e", qs=4)
                rsum = nrmpool.tile([128, 4, 1], F32, tag="rs", name="rsum")
                nc.vector.tensor_copy(rsum[:, :, :], otv[:, :, 64:65])
                rinv = nrmpool.tile([128, 4, 1], F32, tag="ri", name="rinv")
                nc.vector.reciprocal(rinv[:, :, :], rsum[:, :, :])
                otq = oqpool.tile([128, 256], BF16, tag=f"oq{h}",
                                  name="otq")
                nc.vector.tensor_tensor(
                    otq.rearrange("p (qs e) -> p qs e", qs=4),
                    otv[:, :, 0:64], _bcast_inner(rinv, 4, 64),
                    mybir.AluOpType.mult)
                pair.append(otq)
            otqs[(qc, hp)] = pair

        def epi_transpose(qc, hp):
            # PE transposes (identity matmul) + DVE copy psum -> otn
            pair = otqs.pop((qc, hp))
            trans = trpool.tile([128, 512], F32, tag="tr", name="trans")
            for h in range(2):
                otq = pair[h]
                for qs in range(4):
                    nc.tensor.matmul(
                        trans[h * 64:(h + 1) * 64,
                              qs * 128:(qs + 1) * 128],
                        otq[:, qs * 64:(qs + 1) * 64], ident_sb[:, :],
                        start=(qs == 0), stop=(qs == 3))
            nc.vector.tensor_copy(otn_sb[hp][:, qc * 512:(qc + 1) * 512],
                                  trans[:, :])

        LEAD = 4
        TDEFER = 3   # steps between epi_norm and epi_transpose
        # JIT projection schedules: V chunk (kt, hp) must land before
        # s2 needs vp[kt] at g = 16*hp + kt (first sweep, qc0); Q chunk
        # (tq, ct) before s1 reads qpt[ct][tq] at g = 64*tq + 16*ct.
        v_sched = {}

        def _vsched(g, kt2, vhp):
            if g % 16 == 15:     # keep epilogue steps free
                g += 1
            v_sched.setdefault(g, []).append((kt2, vhp))

        for kt2 in range(2, NKT, 2):     # pairs (kt, kt+1)
            _vsched(kt2 - 2, kt2, 0)
        for kt2 in range(0, NKT, 2):
            _vsched(kt2 + 3, kt2, 1)
            _vsched(kt2 + 21, kt2, 2)
            _vsched(kt2 + 39, kt2, 3)
        q_sched = {3: (0, 1), 7: (0, 2), 11: (0, 3)}
        xq_sched = {}
        for tq in (1, 2, 3):
            xq_sched[64 * (tq - 1) + 45] = tq
            for ct in range(NCT):
                # kt == 1 steps: clear of the D matmuls on kt 8-15
                q_sched[64 * tq + 16 * ct - 15] = (tq, ct)
        # warmup: V inputs + first mask load, lead score tiles, V pair
        _load_vtm(0, 0)
        _load_mask(0)
        s1(0)
        s1(1)
        s1(2)
        s1(3)
        emit_v(0, 0)
        for g in range(NG):
            if g + LEAD < NG:
                s1(g + LEAD)
            s2(g)
            qc, hp, ktile = _coords(g)
            if g == 40:      # wo needed from the first D block (g ~ 72)
                nc.gpsimd.dma_start(
                    wo_all.rearrange("p (c j) -> p c j", c=NCT),
                    wot.rearrange("(c p) j -> p c j", p=128))
            if qc > 0 and ktile >= 8:
                # previous qcol's output projection, one matmul per step
                # (kt 8..15) so no step overruns the exp pace
                jb = hp * 2 + (ktile - 8) // 4
                ct = (ktile - 8) % 4
                if ct == 0:
                    dstate["ps"] = pypool.tile([128, 512], F32, tag="py",
                                               name="psy")
                    if ktile == 8:
                        dstate["ye"] = ypool.tile([128, 2 * 512], F32,
                                                  tag="ye", name="ye")
                nc.tensor.matmul(
                    dstate["ps"][:, :],
                    wo_all[:, ct * D + jb * 128:ct * D + (jb + 1) * 128],
                    otn_sb[ct][:, (qc - 1) * 512:qc * 512],
                    start=(ct == 0), stop=(ct == NCT - 1))
                if ct == NCT - 1:
                    nc.vector.tensor_copy(
                        dstate["ye"][:, (jb % 2) * 512:(jb % 2 + 1) * 512],
                        dstate["ps"][:, :])
                    if ktile == NKT - 1:
                        _emit_d_store(nc, dstate["ye"], yt, qc - 1, hp)
            if ktile == NKT - 1:
                epi_norm(qc, hp)
            if ktile == TDEFER - 1 and g >= NKT:
                pq, ph = _coords(g - TDEFER - (NKT - 1))[:2]
                epi_transpose(pq, ph)
            for kt2, vhp in v_sched.get(g, ()):
                emit_v(kt2, vhp)
            if g in xq_sched:
                _load_xq(xq_sched[g])
            if g in q_sched:
                emit_q_chunk(*q_sched[g])
        # Tail: final transpose, then the last qcol's 8 projection blocks
        # fully pipelined through 7 psum banks with direct psum->dram
        # stores (no intermediate sbuf copies).
        epi_transpose(NQC - 1, NCT - 1)
        trpool.release()
        otpool.release()
        stpool.release()
        dpool = tc.alloc_tile_pool(name="dtail", bufs=7, space="PSUM")
        qcl = NQC - 1
        ysink = yt.rearrange("(jb p) t -> p jb t", p=128)
        for jb in range(8):
            ps = dpool.tile([128, 512], F32, tag="d", name="psy")
            for ct in range(NCT):
                nc.tensor.matmul(
                    ps[:, :],
                    wo_all[:, ct * D + jb * 128:ct * D + (jb + 1) * 128],
                    otn_sb[ct][:, qcl * 512:(qcl + 1) * 512],
                    start=(ct == 0), stop=(ct == NCT - 1))
            ye = ypool.tile([128, 512], F32, tag=f"yd{jb % 2}", name="yed")
            if jb % 2:      # split evacuation across DVE and ACT
                nc.vector.tensor_copy(ye[:, :], ps[:, :])
            else:
                nc.scalar.activation(ye[:, :], ps[:, :],
                                     mybir.ActivationFunctionType.Copy)
            nc.sync.dma_start(
                ysink[:, jb:jb + 1, qcl * 512:(qcl + 1) * 512],
                ye.rearrange("p (o t) -> p o t", o=1))
        dpool.release()
    pypool.release()


def kernel(q, k, v, mask, Wq, bq, Wk, bk, Wv, bv, Wo, bo, _trace=False):
    if "nc" not in _CACHED:
        _CACHED["nc"] = _build_nc()
    nc = _CACHED["nc"]

    q = np.asarray(q, np.float32)
    k = np.asarray(k, np.float32)
    v = np.asarray(v, np.float32)
    Wq = np.asarray(Wq, np.float32)
    Wk = np.asarray(Wk, np.float32)
    Wv = np.asarray(Wv, np.float32)
    Wo = np.asarray(Wo, np.float32)
    mask = np.asarray(mask)
    ident = np.eye(128, dtype=np.float32).astype(ml_dtypes.bfloat16)

    in_maps = []
    for core in range(8):
        b, g = divmod(core, 2)
        csl = slice(g * C, (g + 1) * C)
        im = {
            "qt": np.ascontiguousarray(q[b].T).astype(ml_dtypes.bfloat16),
            "kt": np.ascontiguousarray(k[b].T).astype(ml_dtypes.bfloat16),
            "vt": np.ascontiguousarray(v[b].T).astype(ml_dtypes.bfloat16),
            "wqt": np.ascontiguousarray((Wq[csl, :] / 8.0).T).astype(ml_dtypes.bfloat16),
            "wkt": np.ascontiguousarray(Wk[csl, :].T).astype(ml_dtypes.bfloat16),
            "wvt": np.ascontiguousarray(Wv[csl, :].T).astype(ml_dtypes.bfloat16),
            "wot": np.ascontiguousarray(Wo[:, csl].T).astype(
                ml_dtypes.bfloat16),
            "maskt": np.ascontiguousarray(
                (~mask[b, 0]).T.astype(np.float32)).astype(ml_dtypes.bfloat16),
            "bqt": np.ascontiguousarray(
                (np.asarray(bq, np.float32)[csl] / 8.0).reshape(C, 1)),
            "bkt": np.ascontiguousarray(
                np.asarray(bk, np.float32)[csl].reshape(C, 1)),
            "identt": ident,
        }
        in_maps.append(im)

    res = bass_utils.run_bass_kernel_spmd(
        nc, in_maps, core_ids=list(range(8)), trace=_trace)
    if _trace:
        _CACHED["last_results"] = res
    outs = [r["yt"] for r in res.results]

    y = np.empty((B, T, D), np.float32)
    const = (Wo @ np.asarray(bv, np.float32)
             + np.asarray(bo, np.float32)).astype(np.float32)
    for b in range(B):
        y[b] = (outs[2 * b] + outs[2 * b + 1]).T + const
    return y


# revision 66
# speedup vs baseline: 1.2045x; 1.0006x over previous
"""Trainium2 Bass kernel for masked multi-head attention.

Reference computation (B=4, T=2048, D=1024, H=16, dh=64):
    qp = q @ Wq.T + bq ; kp = k @ Wk.T + bk ; vp = v @ Wv.T + bv
    s  = (qh @ khT) / 8 ; s = where(mask, -1e6, s) ; p = softmax(s)
    o  = p @ vh ; y = o @ Wo.T + bo

Sharding: 8 cores = (batch b in 0..3) x (head-group g in 0..1).
Each core handles batch b and 8 heads (512 channels), computes a partial
y^T (output projection over its 512 channels); host sums core pairs,
transposes, and adds the bias terms.

Per-core device algorithm (everything in transposed "T-major" layouts so
no on-device transposes are needed):
  A) qpT[c,t] = sum_m WqT[m,c] * qT[m,t]   (float32r matmuls, psum acc)
     kpT likewise.  1/8 score scale folded into WqT host-side.
  B) vp[t,c]  = sum_m vT[m,t] * WvT[m,c]   (untransposed; stored bf16 in
     a [t, 8*65] layout with a ones-column per head for row-sums)
  C) per (qcol, head): ST[k,q] = khT.T @ qhT (f32r) -> exp on ACT (bf16)
     -> multiply by maskT tile (DVE or GPSIMD, broadcast AP over both
     heads) -> transposed PV: OT[q,65] += ptm[k,q].T @ vp[k,65] per
     (head, q-subtile) with col 64 = row sums (ones column).
     Epilogue: recip rowsums (DVE), broadcast-multiply normalize (DVE),
     PE transpose via identity matmul -> [c,q] psum, DVE copy to otn.
  D) yT[j,t] = sum_c wot[c,j] * otn[c,t]   (bf16) -> DMA out.
"""

import sys
import numpy as np

for _p in ("/opt/trn_rl_repo",):
    if _p not in sys.path:
        sys.path.insert(0, _p)

import ml_dtypes
from contextlib import ExitStack

import concourse.bass as bass
import concourse.tile as tile
from concourse import bacc, mybir
from concourse import bass_utils

B, T, D, H = 4, 2048, 1024, 16
DH = 64          # head dim
HC = 8           # heads per core
C = HC * DH      # 512 channels per core
F32 = mybir.dt.float32
F32R = mybir.dt.float32r
BF16 = mybir.dt.bfloat16
EXP = mybir.ActivationFunctionType.Exp

_CACHED = {}
GP_MASK_MOD = 10 ** 9   # g % GP_MASK_MOD == 1 -> mask multiply on gpsimd


def _build_nc():
    nc = bacc.Bacc("TRN2", target_bir_lowering=False, debug=False,
                   enable_asserts=False)
    qt = nc.dram_tensor("qt", [D, T], BF16, kind="ExternalInput").ap()
    kt = nc.dram_tensor("kt", [D, T], BF16, kind="ExternalInput").ap()
    vt = nc.dram_tensor("vt", [D, T], BF16, kind="ExternalInput").ap()
    wqt = nc.dram_tensor("wqt", [D, C], BF16, kind="ExternalInput").ap()
    wkt = nc.dram_tensor("wkt", [D, C], BF16, kind="ExternalInput").ap()
    wvt = nc.dram_tensor("wvt", [D, C], BF16, kind="ExternalInput").ap()
    wot = nc.dram_tensor("wot", [C, D], BF16, kind="ExternalInput").ap()
    maskt = nc.dram_tensor("maskt", [T, T], BF16, kind="ExternalInput").ap()
    bqt = nc.dram_tensor("bqt", [C, 1], F32, kind="ExternalInput").ap()
    bkt = nc.dram_tensor("bkt", [C, 1], F32, kind="ExternalInput").ap()
    identt = nc.dram_tensor("identt", [128, 128], BF16,
                            kind="ExternalInput").ap()
    yt = nc.dram_tensor("yt", [D, T], F32, kind="ExternalOutput").ap()

    with tile.TileContext(nc) as tc, ExitStack() as ctx:
        _emit(ctx, tc, qt, kt, vt, wqt, wkt, wvt, wot, maskt, bqt, bkt,
              identt, yt)
    nc.compile()
    return nc


def _bcast(ap, reps, inner):
    """Repeat `ap`'s last `inner` elements `reps` times: [p, reps, inner]
    with a 0-stride outer dim."""
    return bass.AP(ap.tensor, ap.offset, [ap.ap[0], [0, reps], [1, inner]])


def _bcast_inner(ap, outer, reps):
    """Broadcast each of `ap`'s `outer` elements `reps` times:
    [p, outer, reps] with a 0-stride inner dim."""
    return bass.AP(ap.tensor, ap.offset, [ap.ap[0], [1, outer], [0, reps]])


def _emit_d_jbs(nc, pypool, ye, wo_all, otn_sb, qc, jbs, tag="py"):
    NCT = C // 128
    for jb in jbs:
        ps = pypool.tile([128, 512], F32, tag=tag, name="psy")
        for ct in range(NCT):
            lhs = wo_all[:, ct * D + jb * 128:ct * D + (jb + 1) * 128]
            rhs = otn_sb[ct][:, qc * 512:(qc + 1) * 512]
            nc.tensor.matmul(ps[:, :], lhs, rhs,
                             start=(ct == 0), stop=(ct == NCT - 1))
        nc.vector.tensor_copy(ye[:, (jb % 2) * 512:(jb % 2 + 1) * 512],
                              ps[:, :])


def _emit_d_store(nc, ye, yt, qc, pair):
    # gpsimd (swdge) queue: store DMAs wait on DVE ye-copies and must not
    # block input loads queued behind them on the SP queue.
    nc.gpsimd.dma_start(
        yt.rearrange("(jb p) t -> p jb t",
                     p=128)[:, pair * 2:(pair + 1) * 2,
                            qc * 512:(qc + 1) * 512],
        ye.rearrange("p (jb t) -> p jb t", jb=2))


def _emit(ctx, tc, qt, kt, vt, wqt, wkt, wvt, wot, maskt, bqt, bkt, identt,
          yt):
    nc = tc.nc
    NKT = T // 128      # 16 k-tiles
    NQC = 4             # q columns of 512
    NCT = C // 128      # 4 channel tiles (= head pairs)
    NM = D // 128       # 8 contraction tiles

    # ---- persistent SBUF arrays -------------------------------------
    persist = ctx.enter_context(tc.tile_pool(name="persist", bufs=1))
    qpt_sb = [persist.tile([128, T], BF16, tag=f"qpt{i}", name=f"qpt{i}")
              for i in range(NCT)]
    kpt_sb = [persist.tile([128, T], BF16, tag=f"kpt{i}", name=f"kpt{i}")
              for i in range(NCT)]
    VPW = HC * (DH + 1)  # 520
    vp_ext = persist.tile([128, NKT * VPW], BF16, tag="vpext", name="vpext")
    otn_sb = [persist.tile([128, T], BF16, tag=f"otn{i}", name=f"otn{i}")
              for i in range(NCT)]
    bias_sb = persist.tile([128, 2 * NCT], F32, tag="bias", name="bias")
    wo_all = persist.tile([128, NCT * D], BF16, tag="wo", name="wo_all")
    ident_sb = persist.tile([128, 128], BF16, tag="ident", name="ident_sb")

    # mask tiles double-buffered; qc0 mask first so attention never waits.
    mpool = ctx.enter_context(tc.tile_pool(name="mask", bufs=2))
    mask_tiles = {}
    msrc = maskt.rearrange("(kt p) q -> p kt q", p=128)

    def _load_mask(qc):
        # Later masks use the ACT dge queue: a mask load WAR-waits on DVE
        # mults of the retiring tile and must not block input loads behind
        # it on the SP queue.  qc0 (no WAR) stays on SP for strict order.
        m_all = mpool.tile([128, NKT * 512], BF16, tag="m", name="m_all")
        eng = nc.sync if qc == 0 else nc.gpsimd
        eng.dma_start(m_all.rearrange("p (kt q) -> p kt q", kt=NKT),
                      msrc[:, :, qc * 512:(qc + 1) * 512])
        mask_tiles[qc] = m_all

    nc.gpsimd.memset(vp_ext[:, :], 1.0)

    # PSUM: pproj(4, released after A) + py(1) | st(4) + ot(2) + trans(1).
    pypool = tc.alloc_tile_pool(name="py", bufs=1, space="PSUM",
                                side="right")
    ppool = tc.alloc_tile_pool(name="pproj", bufs=1, space="PSUM")

    wpoolq = ctx.enter_context(tc.tile_pool(name="wtsq", bufs=1))
    wq_all = wpoolq.tile([128, NM * C], BF16, tag="wq", name="wq_all")
    wpool2 = ctx.enter_context(tc.tile_pool(name="wts2", bufs=1))
    wv_all = wpool2.tile([128, NM * C], BF16, tag="wv", name="wv")
    vtpool = ctx.enter_context(tc.tile_pool(name="vtin", bufs=8))
    vtm_tiles = {}   # (tg, hp) -> [8 input tiles]
    xq_tiles = {}    # tq -> [8 input tiles]

    qsrc = qt.rearrange("(m p) t -> p m t", p=128)
    vsrc = vt.rearrange("(m p) t -> p m t", p=128)

    def _load_xq(tq):
        if tq in xq_tiles:
            return
        xqb = vtpool.tile([128, NM, 512], BF16, tag="xq", bufs=1,
                          name="xqb")
        nc.sync.dma_start(xqb[:, :, :],
                          qsrc[:, :, tq * 512:(tq + 1) * 512])
        xq_tiles.clear()
        xq_tiles[tq] = xqb

    def emit_q_chunk(tq, ct):
        # projects qpt[:, tq-quarter] for head-pair ct (8 matmuls, 1 bank)
        _load_xq(tq)
        xqb = xq_tiles[tq]
        ps = pypool.tile([128, 512], F32, tag="py", name="pvq")
        for m in range(NM):
            lhs = wq_all[:, m * C + ct * 128:m * C + (ct + 1) * 128]
            nc.tensor.matmul(ps[:, :], lhs, xqb[:, m, :],
                             start=(m == 0), stop=(m == NM - 1))
        nc.vector.tensor_scalar_add(
            qpt_sb[ct][:, tq * 512:(tq + 1) * 512], ps[:, :],
            bias_sb[:, ct:ct + 1])

    def _load_vtm(tg, hp=0):
        # cached per t-group (same data for every head-pair)
        if tg in vtm_tiles or not 0 <= tg < 4:
            return
        vtb = vtpool.tile([128, NM, 512], BF16, tag="vt", bufs=4,
                          name="vtb")
        nc.sync.dma_start(vtb[:, :, :],
                          vsrc[:, :, tg * 512:(tg + 1) * 512])
        vtm_tiles[tg] = vtb

    def emit_v(kt, hp):
        # V projection pair: head-pair hp, t-blocks kt and kt+1 (16
        # matmuls N=128 + one copy -> one psum-bank cycle per 2 steps)
        tg = kt // 4
        _load_vtm(tg)
        if kt % 4 == 0:     # prefetch the next t-group's inputs
            _load_vtm(tg + 1)
        vtb = vtm_tiles[tg]
        ps = pypool.tile([128, 512], F32, tag="py", name="pv")
        for j in range(2):
            i = kt % 4 + j
            for m in range(NM):
                nc.tensor.matmul(
                    ps[:, j * 128:(j + 1) * 128],
                    vtb[:, m, i * 128:(i + 1) * 128],
                    wv_all[:, m * C + hp * 128:m * C + (hp + 1) * 128],
                    start=(j == 0 and m == 0),
                    stop=(j == 1 and m == NM - 1))
        pstr = ps.ap[0][0]
        vstr = vp_ext.ap[0][0]
        dstv = bass.AP(vp_ext.tensor,
                       vp_ext.offset + kt * VPW + hp * 130,
                       [[vstr, 128], [VPW, 2], [65, 2], [1, DH]])
        srcv = bass.AP(ps.tensor, ps.offset,
                       [[pstr, 128], [128, 2], [64, 2], [1, DH]])
        nc.vector.tensor_copy(dstv, srcv)

    # ---- Phase A: K projection; q0/hp0 comes via emit_q_chunk -------
    warm = persist.tile([1, 2], F32, tag="warm", name="warm")
    with tc.tile_pool(name="wts", bufs=1) as wpool, \
         tc.tile_pool(name="xin", bufs=4) as xpool:
        wk_all = wpool.tile([128, NM * C], BF16, tag="w", name="wk_all")
        ksrc = kt.rearrange("(m p) t -> p m t", p=128)
        wksrc = wkt.rearrange("(m p) c -> p m c", p=128)
        wkv = wk_all.rearrange("p (m c) -> p m c", m=NM)
        # first two m-blocks of weights+inputs lead so matmuls start ~2us
        nc.sync.dma_start(wkv[:, 0:2], wksrc[:, 0:2])
        xall = {}
        for th in (0, 1):
            for mh in range(2):
                xmb = xpool.tile([128, NM // 2, 1024], BF16, tag="x",
                                 name="x")
                if th == 0 and mh == 0:
                    nc.sync.dma_start(xmb[:, 0:2, :],
                                      ksrc[:, 0:2, 0:1024])
                    nc.sync.dma_start(wkv[:, 2:4], wksrc[:, 2:4])
                    nc.sync.dma_start(xmb[:, 2:4, :],
                                      ksrc[:, 2:4, 0:1024])
                    nc.sync.dma_start(
                        bias_sb[:, NCT:2 * NCT],
                        bkt.rearrange("(c p) o -> p (c o)", p=128))
                    nc.sync.dma_start(wkv[:, 4:NM], wksrc[:, 4:NM])
                    nc.sync.dma_start(
                        bias_sb[:, 0:NCT],
                        bqt.rearrange("(c p) o -> p (c o)", p=128))
                    nc.sync.dma_start(ident_sb[:, :], identt)
                    nc.sync.dma_start(
                        wq_all.rearrange("p (m c) -> p m c", m=NM),
                        wqt.rearrange("(m p) c -> p m c", p=128))
                    _load_xq(0)
                else:
                    nc.sync.dma_start(
                        xmb[:, :, :],
                        ksrc[:, mh * 4:(mh + 1) * 4,
                             th * 1024:(th + 1) * 1024])
                xall[(th, mh)] = xmb
        # warm the ACT exp table while DMAs stream
        nc.gpsimd.memset(warm[:, :], 0.0)
        nc.scalar.activation(warm[:, :], warm[:, :], EXP)
        # v weights follow the k/q input stream
        nc.sync.dma_start(wv_all.rearrange("p (m c) -> p m c", m=NM),
                          wvt.rearrange("(m p) c -> p m c", p=128))
        for th in (0, 1):               # halves of T
            xh = [xall[(th, 0)], xall[(th, 1)]]
            for tc2 in (0, 1):
                psq = {}
                npass = th * 2 + tc2
                for m in range(NM):
                    for ct in range(NCT):
                        lhs = wk_all[:, m * C + ct * 128:
                                     m * C + (ct + 1) * 128]
                        if m == 0:
                            # rotate over 7 banks so pass N+1 never
                            # WAR-waits on pass N's bias-add
                            tg7 = (npass * 4 + ct) % 7
                            psq[ct] = ppool.tile(
                                [128, 512], F32, tag=f"pp{tg7}",
                                name=f"pp{tg7}")
                        nc.tensor.matmul(
                            psq[ct][:, :],
                            lhs,
                            xh[m // 4][:, m % 4,
                                       tc2 * 512:(tc2 + 1) * 512],
                            start=(m == 0), stop=(m == NM - 1))
                for ct in range(NCT):
                    tq = th * 2 + tc2
                    nc.vector.tensor_scalar_add(
                        kpt_sb[ct][:, tq * 512:(tq + 1) * 512],
                        psq[ct][:, :],
                        bias_sb[:, NCT + ct:NCT + ct + 1])
                if th == 0 and tc2 == 1:
                    # q0/hp0 here: its bias-add lands while DVE is free,
                    # so the warmup scores are not gated by K's adds.
                    emit_q_chunk(0, 0)
    ppool.release()

    # ---- Phase C: attention, with phase-D block interleaved per qc --
    stpool = tc.alloc_tile_pool(name="st", bufs=2, space="PSUM",
                                side="right")
    otpool = tc.alloc_tile_pool(name="ot", bufs=1, space="PSUM",
                                side="right")
    trpool = tc.alloc_tile_pool(name="tr", bufs=1, space="PSUM",
                                side="right")
    with tc.tile_pool(name="pt", bufs=5) as ptpool, \
         tc.tile_pool(name="ptm", bufs=4) as ptmpool, \
         tc.tile_pool(name="nrm", bufs=2) as nrmpool, \
         tc.tile_pool(name="oq", bufs=2) as oqpool, \
         tc.tile_pool(name="yev", bufs=2) as ypool:
        NG = NQC * NCT * NKT          # 256 pipeline steps
        ptms = {}                     # g -> masked-prob tile
        ots = {}                      # (qc, hp) -> [ot_h0, ot_h1]
        dstate = {}                   # rolling phase-D psum/ye tiles

        def _coords(g):
            qc, r = divmod(g, NCT * NKT)
            hp, ktile = divmod(r, NKT)
            return qc, hp, ktile

        def s1(g):
            # QK^T -> exp -> mask multiply for step g (LEAD ahead of PV)
            qc, hp, ktile = _coords(g)
            if hp == 1 and ktile == 0 and qc + 1 < NQC \
                    and qc + 1 not in mask_tiles:
                _load_mask(qc + 1)
            m_all = mask_tiles[qc]
            st = stpool.tile([128, 1024], F32, tag="st", name="st")
            for h in range(2):
                lhs = kpt_sb[hp][h * 64:(h + 1) * 64,
                                 ktile * 128:(ktile + 1) * 128]
                rhs = qpt_sb[hp][h * 64:(h + 1) * 64,
                                 qc * 512:(qc + 1) * 512]
                nc.tensor.matmul(st[:, h * 512:(h + 1) * 512],
                                 lhs, rhs, start=True, stop=True)
            pt = ptpool.tile([128, 1024], BF16, tag="pt", name="pt")
            nc.scalar.activation(pt[:, :], st[:, :], EXP)
            ptm = ptmpool.tile([128, 1024], BF16, tag="ptm", name="ptm")
            msl = m_all[:, ktile * 512:(ktile + 1) * 512]
            eng = nc.gpsimd if (g % GP_MASK_MOD == 1) else nc.vector
            eng.tensor_tensor(
                ptm.rearrange("p (t q) -> p t q", t=2),
                pt.rearrange("p (t q) -> p t q", t=2),
                _bcast(msl, 2, 512), mybir.AluOpType.mult)
            ptms[g] = ptm

        def s2(g):
            # transposed PV accumulation for step g
            qc, hp, ktile = _coords(g)
            if ktile == 0:
                ots[(qc, hp)] = [otpool.tile([128, 260], F32, tag=f"ot{h}",
                                             name=f"ot{h}")
                                 for h in range(2)]
            ot2 = ots[(qc, hp)]
            ptm = ptms.pop(g)
            for h in range(2):
                hg = hp * 2 + h
                vsl = vp_ext[:, ktile * VPW + hg * 65:
                             ktile * VPW + (hg + 1) * 65]
                for qs in range(4):
                    # one psum group per bank: the first matmul's start
                    # marks the whole zero region pending-zero, later
                    # slices replace-then-accumulate (has_written bits)
                    nc.tensor.matmul(
                        ot2[h][:, qs * 65:(qs + 1) * 65],
                        ptm[:, h * 512 + qs * 128:h * 512 + (qs + 1) * 128],
                        vsl,
                        start=(ktile == 0 and qs == 0),
                        stop=(ktile == NKT - 1 and qs == 3))

        otqs = {}                     # (qc, hp) -> [otq_h0, otq_h1]

        def epi_norm(qc, hp):
            # DVE-only: gather row sums, reciprocal, broadcast-multiply
            ot2 = ots.pop((qc, hp))
            pair = []
            for h in range(2):
                otv = ot2[h].rearrange("p (qs e) -> p qs e", qs=4)
                rsum = nrmpool.tile([128, 4, 1], F32, tag="rs", name="rsum")
                nc.vector.tensor_copy(rsum[:, :, :], otv[:, :, 64:65])
                rinv = nrmpool.tile([128, 4, 1], F32, tag="ri", name="rinv")
                nc.vector.reciprocal(rinv[:, :, :], rsum[:, :, :])
                otq = oqpool.tile([128, 256], BF16, tag=f"oq{h}",
                                  name="otq")
                nc.vector.tensor_tensor(
                    otq.rearrange("p (qs e) -> p qs e", qs=4),
                    otv[:, :, 0:64], _bcast_inner(rinv, 4, 64),
                    mybir.AluOpType.mult)
                pair.append(otq)
            otqs[(qc, hp)] = pair

        def epi_transpose(qc, hp):
            # PE transposes (identity matmul) + DVE copy psum -> otn
            pair = otqs.pop((qc, hp))
            trans = trpool.tile([128, 512], F32, tag="tr", name="trans")
            for h in range(2):
                otq = pair[h]
                for qs in range(4):
                    nc.tensor.matmul(
                        trans[h * 64:(h + 1) * 64,
                              qs * 128:(qs + 1) * 128],
                        otq[:, qs * 64:(qs + 1) * 64], ident_sb[:, :],
                        start=(qs == 0), stop=(qs == 3))
            nc.vector.tensor_copy(otn_sb[hp][:, qc * 512:(qc + 1) * 512],
                                  trans[:, :])

        LEAD = 4
        TDEFER = 3   # steps between epi_norm and epi_transpose
        # JIT projection schedules: V chunk (kt, hp) must land before
        # s2 needs vp[kt] at g = 16*hp + kt (first sweep, qc0); Q chunk
        # (tq, ct) before s1 reads qpt[ct][tq] at g = 64*tq + 16*ct.
        v_sched = {}

        def _vsched(g, kt2, vhp):
            if g % 16 == 15:     # keep epilogue steps free
                g += 1
            v_sched.setdefault(g, []).append((kt2, vhp))

        for kt2 in range(2, NKT, 2):     # pairs (kt, kt+1)
            _vsched(kt2 - 2, kt2, 0)
        for kt2 in range(0, NKT, 2):
            _vsched(kt2 + 3, kt2, 1)
            _vsched(kt2 + 21, kt2, 2)
            _vsched(kt2 + 39, kt2, 3)
        q_sched = {3: (0, 1), 7: (0, 2), 11: (0, 3)}
        xq_sched = {}
        for tq in (1, 2, 3):
            xq_sched[64 * (tq - 1) + 45] = tq
            for ct in range(NCT):
                # kt == 1 steps: clear of the D matmuls on kt 8-15
                q_sched[64 * tq + 16 * ct - 15] = (tq, ct)
        # warmup: V inputs + first mask load, lead score tiles, V pair
        _load_vtm(0, 0)
        _load_mask(0)
        s1(0)
        s1(1)
        s1(2)
        s1(3)
        emit_v(0, 0)
        for g in range(NG):
            if g + LEAD < NG:
                s1(g + LEAD)
            s2(g)
            qc, hp, ktile = _coords(g)
            if g == 40:      # wo needed from the first D block (g ~ 72)
                nc.gpsimd.dma_start(
                    wo_all.rearrange("p (c j) -> p c j", c=NCT),
                    wot.rearrange("(c p) j -> p c j", p=128))
            if qc > 0 and ktile >= 8:
                # previous qcol's output projection, one matmul per step
                # (kt 8..15) so no step overruns the exp pace
                jb = hp * 2 + (ktile - 8) // 4
                ct = (ktile - 8) % 4
                if ct == 0:
                    dstate["ps"] = pypool.tile([128, 512], F32, tag="py",
                                               name="psy")
                    if ktile == 8:
                        dstate["ye"] = ypool.tile([128, 2 * 512], F32,
                                                  tag="ye", name="ye")
                nc.tensor.matmul(
                    dstate["ps"][:, :],
                    wo_all[:, ct * D + jb * 128:ct * D + (jb + 1) * 128],
                    otn_sb[ct][:, (qc - 1) * 512:qc * 512],
                    start=(ct == 0), stop=(ct == NCT - 1))
                if ct == NCT - 1:
                    nc.vector.tensor_copy(
                        dstate["ye"][:, (jb % 2) * 512:(jb % 2 + 1) * 512],
                        dstate["ps"][:, :])
                    if ktile == NKT - 1:
                        _emit_d_store(nc, dstate["ye"], yt, qc - 1, hp)
            if ktile == NKT - 1:
                epi_norm(qc, hp)
            if ktile == TDEFER - 1 and g >= NKT:
                pq, ph = _coords(g - TDEFER - (NKT - 1))[:2]
                epi_transpose(pq, ph)
            for kt2, vhp in v_sched.get(g, ()):
                emit_v(kt2, vhp)
            if g in xq_sched:
                _load_xq(xq_sched[g])
            if g in q_sched:
                emit_q_chunk(*q_sched[g])
        # Tail: final transpose, then the last qcol's 8 projection blocks
        # fully pipelined through 7 psum banks with direct psum->dram
        # stores (no intermediate sbuf copies).
        epi_transpose(NQC - 1, NCT - 1)
        trpool.release()
        otpool.release()
        stpool.release()
        dpool = tc.alloc_tile_pool(name="dtail", bufs=7, space="PSUM")
        qcl = NQC - 1
        ysink = yt.rearrange("(jb p) t -> p jb t", p=128)
        for jb in range(8):
            ps = dpool.tile([128, 512], F32, tag="d", name="psy")
            for ct in range(NCT):
                nc.tensor.matmul(
                    ps[:, :],
                    wo_all[:, ct * D + jb * 128:ct * D + (jb + 1) * 128],
                    otn_sb[ct][:, qcl * 512:(qcl + 1) * 512],
                    start=(ct == 0), stop=(ct == NCT - 1))
            ye = ypool.tile([128, 512], F32, tag=f"yd{jb % 2}", name="yed")
            if jb % 2:      # split evacuation across DVE and ACT
                nc.vector.tensor_copy(ye[:, :], ps[:, :])
            else:
                nc.scalar.activation(ye[:, :], ps[:, :],
                                     mybir.ActivationFunctionType.Copy)
            nc.sync.dma_start(
                ysink[:, jb:jb + 1, qcl * 512:(qcl + 1) * 512],
                ye.rearrange("p (o t) -> p o t", o=1))
        dpool.release()
    pypool.release()


def kernel(q, k, v, mask, Wq, bq, Wk, bk, Wv, bv, Wo, bo, _trace=False):
    if "nc" not in _CACHED:
        _CACHED["nc"] = _build_nc()
    nc = _CACHED["nc"]

    q = np.asarray(q, np.float32)
    k = np.asarray(k, np.float32)
    v = np.asarray(v, np.float32)
    Wq = np.asarray(Wq, np.float32)
    Wk = np.asarray(Wk, np.float32)
    Wv = np.asarray(Wv, np.float32)
    Wo = np.asarray(Wo, np.float32)
    mask = np.asarray(mask)
    ident = np.eye(128, dtype=np.float32).astype(ml_dtypes.bfloat16)

    in_maps = []
    for core in range(8):
        b, g = divmod(core, 2)
        csl = slice(g * C, (g + 1) * C)
        im = {
            "qt": np.ascontiguousarray(q[b].T).astype(ml_dtypes.bfloat16),
            "kt": np.ascontiguousarray(k[b].T).astype(ml_dtypes.bfloat16),
            "vt": np.ascontiguousarray(v[b].T).astype(ml_dtypes.bfloat16),
            "wqt": np.ascontiguousarray((Wq[csl, :] / 8.0).T).astype(ml_dtypes.bfloat16),
            "wkt": np.ascontiguousarray(Wk[csl, :].T).astype(ml_dtypes.bfloat16),
            "wvt": np.ascontiguousarray(Wv[csl, :].T).astype(ml_dtypes.bfloat16),
            "wot": np.ascontiguousarray(Wo[:, csl].T).astype(
                ml_dtypes.bfloat16),
            "maskt": np.ascontiguousarray(
                (~mask[b, 0]).T.astype(np.float32)).astype(ml_dtypes.bfloat16),
            "bqt": np.ascontiguousarray(
                (np.asarray(bq, np.float32)[csl] / 8.0).reshape(C, 1)),
            "bkt": np.ascontiguousarray(
                np.asarray(bk, np.float32)[csl].reshape(C, 1)),
            "identt": ident,
        }
        in_maps.append(im)

    res = bass_utils.run_bass_kernel_spmd(
        nc, in_maps, core_ids=list(range(8)), trace=_trace)
    if _trace:
        _CACHED["last_results"] = res
    outs = [r["yt"] for r in res.results]

    y = np.empty((B, T, D), np.float32)
    const = (Wo @ np.asarray(bv, np.float32)
             + np.asarray(bo, np.float32)).astype(np.float32)
    for b in range(B):
        y[b] = (outs[2 * b] + outs[2 * b + 1]).T + const
    return y


# revision 76
# speedup vs baseline: 1.2059x; 1.0011x over previous
"""Trainium2 Bass kernel for masked multi-head attention.

Reference computation (B=4, T=2048, D=1024, H=16, dh=64):
    qp = q @ Wq.T + bq ; kp = k @ Wk.T + bk ; vp = v @ Wv.T + bv
    s  = (qh @ khT) / 8 ; s = where(mask, -1e6, s) ; p = softmax(s)
    o  = p @ vh ; y = o @ Wo.T + bo

Sharding: 8 cores = (batch b in 0..3) x (head-group g in 0..1).
Each core handles batch b and 8 heads (512 channels), computes a partial
y^T (output projection over its 512 channels); host sums core pairs,
transposes, and adds the bias terms.

Per-core device algorithm (everything in transposed "T-major" layouts so
no on-device transposes are needed):
  A) qpT[c,t] = sum_m WqT[m,c] * qT[m,t]   (float32r matmuls, psum acc)
     kpT likewise.  1/8 score scale folded into WqT host-side.
  B) vp[t,c]  = sum_m vT[m,t] * WvT[m,c]   (untransposed; stored bf16 in
     a [t, 8*65] layout with a ones-column per head for row-sums)
  C) per (qcol, head): ST[k,q] = khT.T @ qhT (f32r) -> exp on ACT (bf16)
     -> multiply by maskT tile (DVE or GPSIMD, broadcast AP over both
     heads) -> transposed PV: OT[q,65] += ptm[k,q].T @ vp[k,65] per
     (head, q-subtile) with col 64 = row sums (ones column).
     Epilogue: recip rowsums (DVE), broadcast-multiply normalize (DVE),
     PE transpose via identity matmul -> [c,q] psum, DVE copy to otn.
  D) yT[j,t] = sum_c wot[c,j] * otn[c,t]   (bf16) -> DMA out.
"""

import sys
import numpy as np

for _p in ("/opt/trn_rl_repo",):
    if _p not in sys.path:
        sys.path.insert(0, _p)

import ml_dtypes
from contextlib import ExitStack

import concourse.bass as bass
import concourse.tile as tile
from concourse import bacc, mybir
from concourse import bass_utils

B, T, D, H = 4, 2048, 1024, 16
DH = 64          # head dim
HC = 8           # heads per core
C = HC * DH      # 512 channels per core
F32 = mybir.dt.float32
F32R = mybir.dt.float32r
BF16 = mybir.dt.bfloat16
EXP = mybir.ActivationFunctionType.Exp

_CACHED = {}
GP_MASK_MOD = 10 ** 9   # g % GP_MASK_MOD == 1 -> mask multiply on gpsimd


def _build_nc():
    nc = bacc.Bacc("TRN2", target_bir_lowering=False, debug=False,
                   enable_asserts=False)
    qt = nc.dram_tensor("qt", [D, T], BF16, kind="ExternalInput").ap()
    kt = nc.dram_tensor("kt", [D, T], BF16, kind="ExternalInput").ap()
    vt = nc.dram_tensor("vt", [D, T], BF16, kind="ExternalInput").ap()
    wqt = nc.dram_tensor("wqt", [D, C], BF16, kind="ExternalInput").ap()
    wkt = nc.dram_tensor("wkt", [D, C], BF16, kind="ExternalInput").ap()
    wvt = nc.dram_tensor("wvt", [D, C], BF16, kind="ExternalInput").ap()
    wot = nc.dram_tensor("wot", [C, D], BF16, kind="ExternalInput").ap()
    maskt = nc.dram_tensor("maskt", [T, T], BF16, kind="ExternalInput").ap()
    bqt = nc.dram_tensor("bqt", [C, 1], F32, kind="ExternalInput").ap()
    bkt = nc.dram_tensor("bkt", [C, 1], F32, kind="ExternalInput").ap()
    identt = nc.dram_tensor("identt", [128, 128], BF16,
                            kind="ExternalInput").ap()
    yt = nc.dram_tensor("yt", [D, T], F32, kind="ExternalOutput").ap()

    with tile.TileContext(nc) as tc, ExitStack() as ctx:
        _emit(ctx, tc, qt, kt, vt, wqt, wkt, wvt, wot, maskt, bqt, bkt,
              identt, yt)
    nc.compile()
    return nc


def _bcast(ap, reps, inner):
    """Repeat `ap`'s last `inner` elements `reps` times: [p, reps, inner]
    with a 0-stride outer dim."""
    return bass.AP(ap.tensor, ap.offset, [ap.ap[0], [0, reps], [1, inner]])


def _bcast_inner(ap, outer, reps):
    """Broadcast each of `ap`'s `outer` elements `reps` times:
    [p, outer, reps] with a 0-stride inner dim."""
    return bass.AP(ap.tensor, ap.offset, [ap.ap[0], [1, outer], [0, reps]])


def _emit_d_jbs(nc, pypool, ye, wo_all, otn_sb, qc, jbs, tag="py"):
    NCT = C // 128
    for jb in jbs:
        ps = pypool.tile([128, 512], F32, tag=tag, name="psy")
        for ct in range(NCT):
            lhs = wo_all[:, ct * D + jb * 128:ct * D + (jb + 1) * 128]
            rhs = otn_sb[ct][:, qc * 512:(qc + 1) * 512]
            nc.tensor.matmul(ps[:, :], lhs, rhs,
                             start=(ct == 0), stop=(ct == NCT - 1))
        nc.vector.tensor_copy(ye[:, (jb % 2) * 512:(jb % 2 + 1) * 512],
                              ps[:, :])


def _emit_d_store(nc, ye, yt, qc, pair):
    # gpsimd (swdge) queue: store DMAs wait on DVE ye-copies and must not
    # block input loads queued behind them on the SP queue.
    nc.gpsimd.dma_start(
        yt.rearrange("(jb p) t -> p jb t",
                     p=128)[:, pair * 2:(pair + 1) * 2,
                            qc * 512:(qc + 1) * 512],
        ye.rearrange("p (jb t) -> p jb t", jb=2))


def _emit(ctx, tc, qt, kt, vt, wqt, wkt, wvt, wot, maskt, bqt, bkt, identt,
          yt):
    nc = tc.nc
    NKT = T // 128      # 16 k-tiles
    NQC = 4             # q columns of 512
    NCT = C // 128      # 4 channel tiles (= head pairs)
    NM = D // 128       # 8 contraction tiles

    # ---- persistent SBUF arrays -------------------------------------
    persist = ctx.enter_context(tc.tile_pool(name="persist", bufs=1))
    qpt_sb = [persist.tile([128, T], BF16, tag=f"qpt{i}", name=f"qpt{i}")
              for i in range(NCT)]
    kpt_sb = [persist.tile([128, T], BF16, tag=f"kpt{i}", name=f"kpt{i}")
              for i in range(NCT)]
    VPW = HC * (DH + 1)  # 520
    vp_ext = persist.tile([128, NKT * VPW], BF16, tag="vpext", name="vpext")
    otn_sb = [persist.tile([128, T], BF16, tag=f"otn{i}", name=f"otn{i}")
              for i in range(NCT)]
    bias_sb = persist.tile([128, 2 * NCT], F32, tag="bias", name="bias")
    wo_all = persist.tile([128, NCT * D], BF16, tag="wo", name="wo_all")
    ident_sb = persist.tile([128, 128], BF16, tag="ident", name="ident_sb")
    warm = persist.tile([1, 2], F32, tag="warm", name="warm")

    # mask tiles double-buffered; qc0 mask first so attention never waits.
    mpool = ctx.enter_context(tc.tile_pool(name="mask", bufs=2))
    mask_tiles = {}
    msrc = maskt.rearrange("(kt p) q -> p kt q", p=128)

    def _load_mask(qc):
        # Later masks use the ACT dge queue: a mask load WAR-waits on DVE
        # mults of the retiring tile and must not block input loads behind
        # it on the SP queue.  qc0 (no WAR) stays on SP for strict order.
        m_all = mpool.tile([128, NKT * 512], BF16, tag="m", name="m_all")
        eng = nc.sync if qc == 0 else nc.gpsimd
        eng.dma_start(m_all.rearrange("p (kt q) -> p kt q", kt=NKT),
                      msrc[:, :, qc * 512:(qc + 1) * 512])
        mask_tiles[qc] = m_all

    nc.gpsimd.memset(vp_ext[:, :], 1.0)

    # PSUM: py(1) + st(4) right, all along; pproj(3, released after A)
    # left, then ot(2) + trans(1) right.
    pypool = tc.alloc_tile_pool(name="py", bufs=1, space="PSUM",
                                side="right")
    stpool = tc.alloc_tile_pool(name="st", bufs=2, space="PSUM",
                                side="right")
    ppool = tc.alloc_tile_pool(name="pproj", bufs=1, space="PSUM")
    ptpool = ctx.enter_context(tc.tile_pool(name="pt", bufs=4))
    ptmpool = ctx.enter_context(tc.tile_pool(name="ptm", bufs=4))
    ptms = {}                     # g -> masked-prob tile

    def _coords(g):
        qc, r = divmod(g, NCT * NKT)
        hp, ktile = divmod(r, NKT)
        return qc, hp, ktile

    def s1(g):
        # QK^T -> exp -> mask multiply for step g (LEAD ahead of PV)
        qc, hp, ktile = _coords(g)
        if hp == 1 and ktile == 0 and qc + 1 < NQC \
                and qc + 1 not in mask_tiles:
            _load_mask(qc + 1)
        m_all = mask_tiles[qc]
        st = stpool.tile([128, 1024], F32, tag="st", name="st")
        for h in range(2):
            nc.tensor.matmul(
                st[:, h * 512:(h + 1) * 512],
                kpt_sb[hp][h * 64:(h + 1) * 64,
                           ktile * 128:(ktile + 1) * 128],
                qpt_sb[hp][h * 64:(h + 1) * 64,
                           qc * 512:(qc + 1) * 512],
                start=True, stop=True)
        pt = ptpool.tile([128, 1024], BF16, tag="pt", name="pt")
        nc.scalar.activation(pt[:, :], st[:, :], EXP)
        ptm = ptmpool.tile([128, 1024], BF16, tag="ptm", name="ptm")
        msl = m_all[:, ktile * 512:(ktile + 1) * 512]
        nc.vector.tensor_tensor(
            ptm.rearrange("p (t q) -> p t q", t=2),
            pt.rearrange("p (t q) -> p t q", t=2),
            _bcast(msl, 2, 512), mybir.AluOpType.mult)
        ptms[g] = ptm

    wpoolq = ctx.enter_context(tc.tile_pool(name="wtsq", bufs=1))
    wq_all = wpoolq.tile([128, NM * C], BF16, tag="wq", name="wq_all")
    wpool2 = ctx.enter_context(tc.tile_pool(name="wts2", bufs=1))
    wv_all = wpool2.tile([128, NM * C], BF16, tag="wv", name="wv")
    vtpool = ctx.enter_context(tc.tile_pool(name="vtin", bufs=8))
    vtm_tiles = {}   # (tg, hp) -> [8 input tiles]
    xq_tiles = {}    # tq -> [8 input tiles]

    qsrc = qt.rearrange("(m p) t -> p m t", p=128)
    vsrc = vt.rearrange("(m p) t -> p m t", p=128)

    def _load_xq(tq):
        if tq in xq_tiles:
            return
        xqb = vtpool.tile([128, NM, 512], BF16, tag="xq", bufs=1,
                          name="xqb")
        nc.sync.dma_start(xqb[:, :, :],
                          qsrc[:, :, tq * 512:(tq + 1) * 512])
        xq_tiles.clear()
        xq_tiles[tq] = xqb

    def emit_q_chunk(tq, ct):
        # projects qpt[:, tq-quarter] for head-pair ct (8 matmuls, 1 bank)
        _load_xq(tq)
        xqb = xq_tiles[tq]
        ps = pypool.tile([128, 512], F32, tag="py", name="pvq")
        for m in range(NM):
            lhs = wq_all[:, m * C + ct * 128:m * C + (ct + 1) * 128]
            nc.tensor.matmul(ps[:, :], lhs, xqb[:, m, :],
                             start=(m == 0), stop=(m == NM - 1))
        nc.vector.tensor_scalar_add(
            qpt_sb[ct][:, tq * 512:(tq + 1) * 512], ps[:, :],
            bias_sb[:, ct:ct + 1])

    def _load_vtm(tg, hp=0):
        # cached per t-group (same data for every head-pair)
        if tg in vtm_tiles or not 0 <= tg < 4:
            return
        vtb = vtpool.tile([128, NM, 512], BF16, tag="vt", bufs=4,
                          name="vtb")
        nc.sync.dma_start(vtb[:, :, :],
                          vsrc[:, :, tg * 512:(tg + 1) * 512])
        vtm_tiles[tg] = vtb

    def emit_v(kt, hp):
        # V projection pair: head-pair hp, t-blocks kt and kt+1 (16
        # matmuls N=128 + one copy -> one psum-bank cycle per 2 steps)
        tg = kt // 4
        _load_vtm(tg)
        if kt % 4 == 0:     # prefetch the next t-group's inputs
            _load_vtm(tg + 1)
        vtb = vtm_tiles[tg]
        ps = pypool.tile([128, 512], F32, tag="py", name="pv")
        for j in range(2):
            i = kt % 4 + j
            for m in range(NM):
                nc.tensor.matmul(
                    ps[:, j * 128:(j + 1) * 128],
                    vtb[:, m, i * 128:(i + 1) * 128],
                    wv_all[:, m * C + hp * 128:m * C + (hp + 1) * 128],
                    start=(j == 0 and m == 0),
                    stop=(j == 1 and m == NM - 1))
        pstr = ps.ap[0][0]
        vstr = vp_ext.ap[0][0]
        dstv = bass.AP(vp_ext.tensor,
                       vp_ext.offset + kt * VPW + hp * 130,
                       [[vstr, 128], [VPW, 2], [65, 2], [1, DH]])
        srcv = bass.AP(ps.tensor, ps.offset,
                       [[pstr, 128], [128, 2], [64, 2], [1, DH]])
        nc.vector.tensor_copy(dstv, srcv)

    # ---- Phase A: K projection; q0/hp0 comes via emit_q_chunk -------
    with tc.tile_pool(name="wts", bufs=1) as wpool, \
         tc.tile_pool(name="xin", bufs=2) as xpool:
        wk_all = wpool.tile([128, NM * C], BF16, tag="w", name="wk_all")
        ksrc = kt.rearrange("(m p) t -> p m t", p=128)
        wksrc = wkt.rearrange("(m p) c -> p m c", p=128)
        wkv = wk_all.rearrange("p (m c) -> p m c", m=NM)
        # first two m-blocks of weights+inputs lead so matmuls start ~2us
        nc.sync.dma_start(wkv[:, 0:2], wksrc[:, 0:2])
        xall = {}
        for th in (0, 1):
            for mh in range(2):
                if th == 1:
                    # stage through the (still empty) vt-cache slots:
                    # later tg loads WAR-wait until pass 4 reads finish
                    xmb = vtpool.tile([128, NM // 2, 1024], BF16,
                                      tag="vt", bufs=4, name="x2")
                else:
                    xmb = xpool.tile([128, NM // 2, 1024], BF16, tag="x",
                                     name="x")
                if th == 0 and mh == 0:
                    nc.sync.dma_start(xmb[:, 0:2, :],
                                      ksrc[:, 0:2, 0:1024])
                    nc.sync.dma_start(wkv[:, 2:4], wksrc[:, 2:4])
                    nc.sync.dma_start(xmb[:, 2:4, :],
                                      ksrc[:, 2:4, 0:1024])
                    nc.sync.dma_start(
                        bias_sb[:, NCT:2 * NCT],
                        bkt.rearrange("(c p) o -> p (c o)", p=128))
                    nc.sync.dma_start(wkv[:, 4:NM], wksrc[:, 4:NM])
                    nc.sync.dma_start(
                        bias_sb[:, 0:NCT],
                        bqt.rearrange("(c p) o -> p (c o)", p=128))
                    nc.sync.dma_start(ident_sb[:, :], identt)
                    nc.sync.dma_start(
                        wq_all.rearrange("p (m c) -> p m c", m=NM),
                        wqt.rearrange("(m p) c -> p m c", p=128))
                    _load_xq(0)
                else:
                    nc.sync.dma_start(
                        xmb[:, :, :],
                        ksrc[:, mh * 4:(mh + 1) * 4,
                             th * 1024:(th + 1) * 1024])
                xall[(th, mh)] = xmb
        # warm the ACT exp table while DMAs stream
        nc.gpsimd.memset(warm[:, :], 0.0)
        nc.scalar.activation(warm[:, :], warm[:, :], EXP)
        # v weights follow the k/q input stream
        nc.sync.dma_start(wv_all.rearrange("p (m c) -> p m c", m=NM),
                          wvt.rearrange("(m p) c -> p m c", p=128))
        for th in (0, 1):               # halves of T
            xh = [xall[(th, 0)], xall[(th, 1)]]
            for tc2 in (0, 1):
                npass = th * 2 + tc2
                tq = th * 2 + tc2
                for ct in range(NCT):   # sequential chains, 3 banks
                    psq = ppool.tile([128, 512], F32,
                                     tag=f"pp{(npass * 4 + ct) % 3}",
                                     name="pp")
                    for m in range(NM):
                        nc.tensor.matmul(
                            psq[:, :],
                            wk_all[:, m * C + ct * 128:
                                   m * C + (ct + 1) * 128],
                            xh[m // 4][:, m % 4,
                                       tc2 * 512:(tc2 + 1) * 512],
                            start=(m == 0), stop=(m == NM - 1))
                    nc.vector.tensor_scalar_add(
                        kpt_sb[ct][:, tq * 512:(tq + 1) * 512],
                        psq[:, :],
                        bias_sb[:, NCT + ct:NCT + ct + 1])
                    if th == 1 and tc2 == 1 and ct < 2:
                        # feed the exp stream between K's last chains
                        s1(2 + ct)
                if th == 0 and tc2 == 1:
                    # q0/hp0 here: its bias-add lands while DVE is free,
                    # so the warmup scores are not gated by K's adds.
                    emit_q_chunk(0, 0)
                if th == 1 and tc2 == 0:
                    # the exp stream starts while K's last pass projects;
                    # exactly two lead scores (= st buffers, no WAR)
                    _load_mask(0)
                    _load_vtm(0)
                    s1(0)
                    s1(1)
    ppool.release()

    # ---- Phase C: attention, with phase-D block interleaved per qc --
    otpool = tc.alloc_tile_pool(name="ot", bufs=1, space="PSUM",
                                side="right")
    trpool = tc.alloc_tile_pool(name="tr", bufs=1, space="PSUM",
                                side="right")
    with tc.tile_pool(name="nrm", bufs=2) as nrmpool, \
         tc.tile_pool(name="oq", bufs=2) as oqpool, \
         tc.tile_pool(name="yev", bufs=2) as ypool:
        NG = NQC * NCT * NKT          # 256 pipeline steps
        ots = {}                      # (qc, hp) -> [ot_h0, ot_h1]
        dstate = {}                   # rolling phase-D psum/ye tiles

        def s2(g):
            # transposed PV accumulation for step g
            qc, hp, ktile = _coords(g)
            if ktile == 0:
                ots[(qc, hp)] = [otpool.tile([128, 260], F32, tag=f"ot{h}",
                                             name=f"ot{h}")
                                 for h in range(2)]
            ot2 = ots[(qc, hp)]
            ptm = ptms.pop(g)
            for h in range(2):
                hg = hp * 2 + h
                vsl = vp_ext[:, ktile * VPW + hg * 65:
                             ktile * VPW + (hg + 1) * 65]
                for qs in range(4):
                    # one psum group per bank: the first matmul's start
                    # marks the whole zero region pending-zero, later
                    # slices replace-then-accumulate (has_written bits)
                    nc.tensor.matmul(
                        ot2[h][:, qs * 65:(qs + 1) * 65],
                        ptm[:, h * 512 + qs * 128:h * 512 + (qs + 1) * 128],
                        vsl,
                        start=(ktile == 0 and qs == 0),
                        stop=(ktile == NKT - 1 and qs == 3))

        otqs = {}                     # (qc, hp) -> [otq_h0, otq_h1]

        def epi_norm(qc, hp):
            # DVE-only: gather row sums, reciprocal, broadcast-multiply
            ot2 = ots.pop((qc, hp))
            pair = []
            for h in range(2):
                otv = ot2[h].rearrange("p (qs e) -> p qs e", qs=4)
                rsum = nrmpool.tile([128, 4, 1], F32, tag="rs", name="rsum")
                nc.vector.tensor_copy(rsum[:, :, :], otv[:, :, 64:65])
                rinv = nrmpool.tile([128, 4, 1], F32, tag="ri", name="rinv")
                nc.vector.reciprocal(rinv[:, :, :], rsum[:, :, :])
                otq = oqpool.tile([128, 256], BF16, tag=f"oq{h}",
                                  name="otq")
                nc.vector.tensor_tensor(
                    otq.rearrange("p (qs e) -> p qs e", qs=4),
                    otv[:, :, 0:64], _bcast_inner(rinv, 4, 64),
                    mybir.AluOpType.mult)
                pair.append(otq)
            otqs[(qc, hp)] = pair

        def epi_transpose(qc, hp):
            # PE transposes (identity matmul) + DVE copy psum -> otn
            pair = otqs.pop((qc, hp))
            trans = trpool.tile([128, 512], F32, tag="tr", name="trans")
            for h in range(2):
                otq = pair[h]
                for qs in range(4):
                    nc.tensor.matmul(
                        trans[h * 64:(h + 1) * 64,
                              qs * 128:(qs + 1) * 128],
                        otq[:, qs * 64:(qs + 1) * 64], ident_sb[:, :],
                        start=(qs == 0), stop=(qs == 3))
            nc.vector.tensor_copy(otn_sb[hp][:, qc * 512:(qc + 1) * 512],
                                  trans[:, :])

        LEAD = 4
        TDEFER = 3   # steps between epi_norm and epi_transpose
        # JIT projection schedules: V chunk (kt, hp) must land before
        # s2 needs vp[kt] at g = 16*hp + kt (first sweep, qc0); Q chunk
        # (tq, ct) before s1 reads qpt[ct][tq] at g = 64*tq + 16*ct.
        v_sched = {}

        def _vsched(g, kt2, vhp):
            if g % 16 == 15:     # keep epilogue steps free
                g += 1
            v_sched.setdefault(g, []).append((kt2, vhp))

        for kt2 in range(2, NKT, 2):     # pairs (kt, kt+1)
            _vsched(kt2 - 2, kt2, 0)
        for kt2 in range(0, NKT, 2):
            _vsched(kt2 + 3, kt2, 1)
            _vsched(kt2 + 21, kt2, 2)
            _vsched(kt2 + 39, kt2, 3)
        q_sched = {3: (0, 1), 7: (0, 2), 11: (0, 3)}
        xq_sched = {}
        for tq in (1, 2, 3):
            xq_sched[64 * (tq - 1) + 45] = tq
            for ct in range(NCT):
                # kt == 1 steps: clear of the D matmuls on kt 8-15
                q_sched[64 * tq + 16 * ct - 15] = (tq, ct)
        # warmup (s1(0..3) came from inside phase A)
        emit_v(0, 0)
        for g in range(NG):
            if g + LEAD < NG:
                s1(g + LEAD)
            s2(g)
            qc, hp, ktile = _coords(g)
            if g == 40:      # wo needed from the first D block (g ~ 72)
                nc.gpsimd.dma_start(
                    wo_all.rearrange("p (c j) -> p c j", c=NCT),
                    wot.rearrange("(c p) j -> p c j", p=128))
            if qc > 0 and ktile >= 8:
                # previous qcol's output projection, one matmul per step
                # (kt 8..15) so no step overruns the exp pace
                jb = hp * 2 + (ktile - 8) // 4
                ct = (ktile - 8) % 4
                if ct == 0:
                    dstate["ps"] = pypool.tile([128, 512], F32, tag="py",
                                               name="psy")
                    if ktile == 8:
                        dstate["ye"] = ypool.tile([128, 2 * 512], F32,
                                                  tag="ye", name="ye")
                nc.tensor.matmul(
                    dstate["ps"][:, :],
                    wo_all[:, ct * D + jb * 128:ct * D + (jb + 1) * 128],
                    otn_sb[ct][:, (qc - 1) * 512:qc * 512],
                    start=(ct == 0), stop=(ct == NCT - 1))
                if ct == NCT - 1:
                    nc.vector.tensor_copy(
                        dstate["ye"][:, (jb % 2) * 512:(jb % 2 + 1) * 512],
                        dstate["ps"][:, :])
                    if ktile == NKT - 1:
                        _emit_d_store(nc, dstate["ye"], yt, qc - 1, hp)
            if ktile == NKT - 1:
                epi_norm(qc, hp)
            if ktile == TDEFER - 1 and g >= NKT:
                pq, ph = _coords(g - TDEFER - (NKT - 1))[:2]
                epi_transpose(pq, ph)
            for kt2, vhp in v_sched.get(g, ()):
                emit_v(kt2, vhp)
            if g in xq_sched:
                _load_xq(xq_sched[g])
            if g in q_sched:
                emit_q_chunk(*q_sched[g])
        # Tail: final transpose, then the last qcol's 8 projection blocks
        # fully pipelined through 7 psum banks with direct psum->dram
        # stores (no intermediate sbuf copies).
        epi_transpose(NQC - 1, NCT - 1)
        trpool.release()
        otpool.release()
        stpool.release()
        dpool = tc.alloc_tile_pool(name="dtail", bufs=7, space="PSUM")
        qcl = NQC - 1
        ysink = yt.rearrange("(jb p) t -> p jb t", p=128)
        for jb in range(8):
            ps = dpool.tile([128, 512], F32, tag="d", name="psy")
            for ct in range(NCT):
                nc.tensor.matmul(
                    ps[:, :],
                    wo_all[:, ct * D + jb * 128:ct * D + (jb + 1) * 128],
                    otn_sb[ct][:, qcl * 512:(qcl + 1) * 512],
                    start=(ct == 0), stop=(ct == NCT - 1))
            ye = ypool.tile([128, 512], F32, tag=f"yd{jb % 2}", name="yed")
            if jb % 2:      # split evacuation across DVE and ACT
                nc.vector.tensor_copy(ye[:, :], ps[:, :])
            else:
                nc.scalar.activation(ye[:, :], ps[:, :],
                                     mybir.ActivationFunctionType.Copy)
            nc.sync.dma_start(
                ysink[:, jb:jb + 1, qcl * 512:(qcl + 1) * 512],
                ye.rearrange("p (o t) -> p o t", o=1))
        dpool.release()
    pypool.release()


def kernel(q, k, v, mask, Wq, bq, Wk, bk, Wv, bv, Wo, bo, _trace=False):
    if "nc" not in _CACHED:
        _CACHED["nc"] = _build_nc()
    nc = _CACHED["nc"]

    q = np.asarray(q, np.float32)
    k = np.asarray(k, np.float32)
    v = np.asarray(v, np.float32)
    Wq = np.asarray(Wq, np.float32)
    Wk = np.asarray(Wk, np.float32)
    Wv = np.asarray(Wv, np.float32)
    Wo = np.asarray(Wo, np.float32)
    mask = np.asarray(mask)
    ident = np.eye(128, dtype=np.float32).astype(ml_dtypes.bfloat16)

    in_maps = []
    for core in range(8):
        b, g = divmod(core, 2)
        csl = slice(g * C, (g + 1) * C)
        im = {
            "qt": np.ascontiguousarray(q[b].T).astype(ml_dtypes.bfloat16),
            "kt": np.ascontiguousarray(k[b].T).astype(ml_dtypes.bfloat16),
            "vt": np.ascontiguousarray(v[b].T).astype(ml_dtypes.bfloat16),
            "wqt": np.ascontiguousarray((Wq[csl, :] / 8.0).T).astype(ml_dtypes.bfloat16),
            "wkt": np.ascontiguousarray(Wk[csl, :].T).astype(ml_dtypes.bfloat16),
            "wvt": np.ascontiguousarray(Wv[csl, :].T).astype(ml_dtypes.bfloat16),
            "wot": np.ascontiguousarray(Wo[:, csl].T).astype(
                ml_dtypes.bfloat16),
            "maskt": np.ascontiguousarray(
                (~mask[b, 0]).T.astype(np.float32)).astype(ml_dtypes.bfloat16),
            "bqt": np.ascontiguousarray(
                (np.asarray(bq, np.float32)[csl] / 8.0).reshape(C, 1)),
            "bkt": np.ascontiguousarray(
                np.asarray(bk, np.float32)[csl].reshape(C, 1)),
            "identt": ident,
        }
        in_maps.append(im)

    res = bass_utils.run_bass_kernel_spmd(
        nc, in_maps, core_ids=list(range(8)), trace=_trace)
    if _trace:
        _CACHED["last_results"] = res
    outs = [r["yt"] for r in res.results]

    y = np.empty((B, T, D), np.float32)
    const = (Wo @ np.asarray(bv, np.float32)
             + np.asarray(bo, np.float32)).astype(np.float32)
    for b in range(B):
        y[b] = (outs[2 * b] + outs[2 * b + 1]).T + const
    return y


# revision 79
# speedup vs baseline: 1.2122x; 1.0052x over previous
"""Trainium2 Bass kernel for masked multi-head attention.

Reference computation (B=4, T=2048, D=1024, H=16, dh=64):
    qp = q @ Wq.T + bq ; kp = k @ Wk.T + bk ; vp = v @ Wv.T + bv
    s  = (qh @ khT) / 8 ; s = where(mask, -1e6, s) ; p = softmax(s)
    o  = p @ vh ; y = o @ Wo.T + bo

Sharding: 8 cores = (batch b in 0..3) x (head-group g in 0..1).
Each core handles batch b and 8 heads (512 channels), computes a partial
y^T (output projection over its 512 channels); host sums core pairs,
transposes, and adds the bias terms.

Per-core device algorithm (everything in transposed "T-major" layouts so
no on-device transposes are needed):
  A) qpT[c,t] = sum_m WqT[m,c] * qT[m,t]   (float32r matmuls, psum acc)
     kpT likewise.  1/8 score scale folded into WqT host-side.
  B) vp[t,c]  = sum_m vT[m,t] * WvT[m,c]   (untransposed; stored bf16 in
     a [t, 8*65] layout with a ones-column per head for row-sums)
  C) per (qcol, head): ST[k,q] = khT.T @ qhT (f32r) -> exp on ACT (bf16)
     -> multiply by maskT tile (DVE or GPSIMD, broadcast AP over both
     heads) -> transposed PV: OT[q,65] += ptm[k,q].T @ vp[k,65] per
     (head, q-subtile) with col 64 = row sums (ones column).
     Epilogue: recip rowsums (DVE), broadcast-multiply normalize (DVE),
     PE transpose via identity matmul -> [c,q] psum, DVE copy to otn.
  D) yT[j,t] = sum_c wot[c,j] * otn[c,t]   (bf16) -> DMA out.
"""

import sys
import numpy as np

for _p in ("/opt/trn_rl_repo",):
    if _p not in sys.path:
        sys.path.insert(0, _p)

import ml_dtypes
from contextlib import ExitStack

import concourse.bass as bass
import concourse.tile as tile
from concourse import bacc, mybir
from concourse import bass_utils

B, T, D, H = 4, 2048, 1024, 16
DH = 64          # head dim
HC = 8           # heads per core
C = HC * DH      # 512 channels per core
F32 = mybir.dt.float32
F32R = mybir.dt.float32r
BF16 = mybir.dt.bfloat16
EXP = mybir.ActivationFunctionType.Exp

_CACHED = {}
GP_MASK_MOD = 10 ** 9   # g % GP_MASK_MOD == 1 -> mask multiply on gpsimd


def _build_nc():
    nc = bacc.Bacc("TRN2", target_bir_lowering=False, debug=False,
                   enable_asserts=False)
    qt = nc.dram_tensor("qt", [D, T], BF16, kind="ExternalInput").ap()
    kt = nc.dram_tensor("kt", [D, T], BF16, kind="ExternalInput").ap()
    vt = nc.dram_tensor("vt", [D, T], BF16, kind="ExternalInput").ap()
    wqt = nc.dram_tensor("wqt", [D, C], BF16, kind="ExternalInput").ap()
    wkt = nc.dram_tensor("wkt", [D, C], BF16, kind="ExternalInput").ap()
    wvt = nc.dram_tensor("wvt", [D, C], BF16, kind="ExternalInput").ap()
    wot = nc.dram_tensor("wot", [C, D], BF16, kind="ExternalInput").ap()
    maskt = nc.dram_tensor("maskt", [T, T], BF16, kind="ExternalInput").ap()
    bqt = nc.dram_tensor("bqt", [C, 1], F32, kind="ExternalInput").ap()
    bkt = nc.dram_tensor("bkt", [C, 1], F32, kind="ExternalInput").ap()
    identt = nc.dram_tensor("identt", [128, 128], BF16,
                            kind="ExternalInput").ap()
    yt = nc.dram_tensor("yt", [D, T], F32, kind="ExternalOutput").ap()

    with tile.TileContext(nc) as tc, ExitStack() as ctx:
        _emit(ctx, tc, qt, kt, vt, wqt, wkt, wvt, wot, maskt, bqt, bkt,
              identt, yt)
    nc.compile()
    return nc


def _bcast(ap, reps, inner):
    """Repeat `ap`'s last `inner` elements `reps` times: [p, reps, inner]
    with a 0-stride outer dim."""
    return bass.AP(ap.tensor, ap.offset, [ap.ap[0], [0, reps], [1, inner]])


def _bcast_inner(ap, outer, reps):
    """Broadcast each of `ap`'s `outer` elements `reps` times:
    [p, outer, reps] with a 0-stride inner dim."""
    return bass.AP(ap.tensor, ap.offset, [ap.ap[0], [1, outer], [0, reps]])


def _emit_d_jbs(nc, pypool, ye, wo_all, otn_sb, qc, jbs, tag="py"):
    NCT = C // 128
    for jb in jbs:
        ps = pypool.tile([128, 512], F32, tag=tag, name="psy")
        for ct in range(NCT):
            lhs = wo_all[:, ct * D + jb * 128:ct * D + (jb + 1) * 128]
            rhs = otn_sb[ct][:, qc * 512:(qc + 1) * 512]
            nc.tensor.matmul(ps[:, :], lhs, rhs,
                             start=(ct == 0), stop=(ct == NCT - 1))
        nc.vector.tensor_copy(ye[:, (jb % 2) * 512:(jb % 2 + 1) * 512],
                              ps[:, :])


def _emit_d_store(nc, ye, yt, qc, pair):
    # gpsimd (swdge) queue: store DMAs wait on DVE ye-copies and must not
    # block input loads queued behind them on the SP queue.
    nc.gpsimd.dma_start(
        yt.rearrange("(jb p) t -> p jb t",
                     p=128)[:, pair * 2:(pair + 1) * 2,
                            qc * 512:(qc + 1) * 512],
        ye.rearrange("p (jb t) -> p jb t", jb=2))


def _emit(ctx, tc, qt, kt, vt, wqt, wkt, wvt, wot, maskt, bqt, bkt, identt,
          yt):
    nc = tc.nc
    NKT = T // 128      # 16 k-tiles
    NQC = 4             # q columns of 512
    NCT = C // 128      # 4 channel tiles (= head pairs)
    NM = D // 128       # 8 contraction tiles

    # ---- persistent SBUF arrays -------------------------------------
    persist = ctx.enter_context(tc.tile_pool(name="persist", bufs=1))
    qpt_sb = [persist.tile([128, T], BF16, tag=f"qpt{i}", name=f"qpt{i}")
              for i in range(NCT)]
    kpt_sb = [persist.tile([128, T], BF16, tag=f"kpt{i}", name=f"kpt{i}")
              for i in range(NCT)]
    VPW = HC * (DH + 1)  # 520
    vp_ext = persist.tile([128, NKT * VPW], BF16, tag="vpext", name="vpext")
    otn_sb = [persist.tile([128, T], BF16, tag=f"otn{i}", name=f"otn{i}")
              for i in range(NCT)]
    bias_sb = persist.tile([128, 2 * NCT], F32, tag="bias", name="bias")
    wo_all = persist.tile([128, NCT * D], BF16, tag="wo", name="wo_all")
    ident_sb = persist.tile([128, 128], BF16, tag="ident", name="ident_sb")
    warm = persist.tile([1, 2], F32, tag="warm", name="warm")

    # mask tiles double-buffered; qc0 mask first so attention never waits.
    mpool = ctx.enter_context(tc.tile_pool(name="mask", bufs=2))
    mask_tiles = {}
    msrc = maskt.rearrange("(kt p) q -> p kt q", p=128)

    def _load_mask(qc):
        # Later masks use the ACT dge queue: a mask load WAR-waits on DVE
        # mults of the retiring tile and must not block input loads behind
        # it on the SP queue.  qc0 (no WAR) stays on SP for strict order.
        m_all = mpool.tile([128, NKT * 512], BF16, tag="m", name="m_all")
        eng = nc.sync if qc == 0 else nc.gpsimd
        eng.dma_start(m_all.rearrange("p (kt q) -> p kt q", kt=NKT),
                      msrc[:, :, qc * 512:(qc + 1) * 512])
        mask_tiles[qc] = m_all

    nc.gpsimd.memset(vp_ext[:, :], 1.0)

    # PSUM: py(1) + st(4) right, all along; pproj(3, released after A)
    # left, then ot(2) + trans(1) right.
    pypool = tc.alloc_tile_pool(name="py", bufs=1, space="PSUM",
                                side="right")
    stpool = tc.alloc_tile_pool(name="st", bufs=2, space="PSUM",
                                side="right")
    ppool = tc.alloc_tile_pool(name="pproj", bufs=1, space="PSUM")
    ptpool = ctx.enter_context(tc.tile_pool(name="pt", bufs=4))
    ptmpool = ctx.enter_context(tc.tile_pool(name="ptm", bufs=4))
    ptms = {}                     # g -> masked-prob tile

    def _coords(g):
        qc, r = divmod(g, NCT * NKT)
        hp, ktile = divmod(r, NKT)
        return qc, hp, ktile

    def s1(g):
        # QK^T -> exp -> mask multiply for step g (LEAD ahead of PV)
        qc, hp, ktile = _coords(g)
        if hp == 1 and ktile == 0 and qc + 1 < NQC \
                and qc + 1 not in mask_tiles:
            _load_mask(qc + 1)
        m_all = mask_tiles[qc]
        st = stpool.tile([128, 1024], F32, tag="st", name="st")
        for h in range(2):
            nc.tensor.matmul(
                st[:, h * 512:(h + 1) * 512],
                kpt_sb[hp][h * 64:(h + 1) * 64,
                           ktile * 128:(ktile + 1) * 128],
                qpt_sb[hp][h * 64:(h + 1) * 64,
                           qc * 512:(qc + 1) * 512],
                start=True, stop=True)
        pt = ptpool.tile([128, 1024], BF16, tag="pt", name="pt")
        nc.scalar.activation(pt[:, :], st[:, :], EXP)
        ptm = ptmpool.tile([128, 1024], BF16, tag="ptm", name="ptm")
        msl = m_all[:, ktile * 512:(ktile + 1) * 512]
        nc.vector.tensor_tensor(
            ptm.rearrange("p (t q) -> p t q", t=2),
            pt.rearrange("p (t q) -> p t q", t=2),
            _bcast(msl, 2, 512), mybir.AluOpType.mult)
        ptms[g] = ptm

    wpoolq = ctx.enter_context(tc.tile_pool(name="wtsq", bufs=1))
    wq_all = wpoolq.tile([128, NM * C], BF16, tag="wq", name="wq_all")
    wpool2 = ctx.enter_context(tc.tile_pool(name="wts2", bufs=1))
    wv_all = wpool2.tile([128, NM * C], BF16, tag="wv", name="wv")
    vtpool = ctx.enter_context(tc.tile_pool(name="vtin", bufs=8))
    vtm_tiles = {}   # (tg, hp) -> [8 input tiles]
    xq_tiles = {}    # tq -> [8 input tiles]

    qsrc = qt.rearrange("(m p) t -> p m t", p=128)
    vsrc = vt.rearrange("(m p) t -> p m t", p=128)

    def _load_xq(tq):
        if tq in xq_tiles:
            return
        xqb = vtpool.tile([128, NM, 512], BF16, tag="xq", bufs=1,
                          name="xqb")
        nc.sync.dma_start(xqb[:, :, :],
                          qsrc[:, :, tq * 512:(tq + 1) * 512])
        xq_tiles.clear()
        xq_tiles[tq] = xqb

    def emit_q_chunk(tq, ct):
        # projects qpt[:, tq-quarter] for head-pair ct (8 matmuls, 1 bank)
        _load_xq(tq)
        xqb = xq_tiles[tq]
        ps = pypool.tile([128, 512], F32, tag="py", name="pvq")
        for m in range(NM):
            lhs = wq_all[:, m * C + ct * 128:m * C + (ct + 1) * 128]
            nc.tensor.matmul(ps[:, :], lhs, xqb[:, m, :],
                             start=(m == 0), stop=(m == NM - 1))
        nc.vector.tensor_scalar_add(
            qpt_sb[ct][:, tq * 512:(tq + 1) * 512], ps[:, :],
            bias_sb[:, ct:ct + 1])

    def _load_vtm(tg, hp=0):
        # cached per t-group (same data for every head-pair)
        if tg in vtm_tiles or not 0 <= tg < 4:
            return
        vtb = vtpool.tile([128, NM, 512], BF16, tag="vt", bufs=4,
                          name="vtb")
        nc.sync.dma_start(vtb[:, :, :],
                          vsrc[:, :, tg * 512:(tg + 1) * 512])
        vtm_tiles[tg] = vtb

    def emit_v(kt, hp):
        # V projection pair: head-pair hp, t-blocks kt and kt+1 (16
        # matmuls N=128 + one copy -> one psum-bank cycle per 2 steps)
        tg = kt // 4
        _load_vtm(tg)
        if kt % 4 == 0:     # prefetch the next t-group's inputs
            _load_vtm(tg + 1)
        vtb = vtm_tiles[tg]
        ps = pypool.tile([128, 512], F32, tag="py", name="pv")
        for j in range(2):
            i = kt % 4 + j
            for m in range(NM):
                nc.tensor.matmul(
                    ps[:, j * 128:(j + 1) * 128],
                    vtb[:, m, i * 128:(i + 1) * 128],
                    wv_all[:, m * C + hp * 128:m * C + (hp + 1) * 128],
                    start=(j == 0 and m == 0),
                    stop=(j == 1 and m == NM - 1))
        pstr = ps.ap[0][0]
        vstr = vp_ext.ap[0][0]
        dstv = bass.AP(vp_ext.tensor,
                       vp_ext.offset + kt * VPW + hp * 130,
                       [[vstr, 128], [VPW, 2], [65, 2], [1, DH]])
        srcv = bass.AP(ps.tensor, ps.offset,
                       [[pstr, 128], [128, 2], [64, 2], [1, DH]])
        nc.vector.tensor_copy(dstv, srcv)

    # ---- Phase A: K projection; q0/hp0 comes via emit_q_chunk -------
    with tc.tile_pool(name="wts", bufs=1) as wpool, \
         tc.tile_pool(name="xin", bufs=2) as xpool:
        wk_all = wpool.tile([128, NM * C], BF16, tag="w", name="wk_all")
        ksrc = kt.rearrange("(m p) t -> p m t", p=128)
        wksrc = wkt.rearrange("(m p) c -> p m c", p=128)
        wkv = wk_all.rearrange("p (m c) -> p m c", m=NM)
        # first two m-blocks of weights+inputs lead so matmuls start ~2us
        nc.sync.dma_start(wkv[:, 0:2], wksrc[:, 0:2])
        xall = {}
        for th in (0, 1):
            for mh in range(2):
                if th == 1:
                    # stage through the (still empty) vt-cache slots:
                    # later tg loads WAR-wait until pass 4 reads finish
                    xmb = vtpool.tile([128, NM // 2, 1024], BF16,
                                      tag="vt", bufs=4, name="x2")
                else:
                    xmb = xpool.tile([128, NM // 2, 1024], BF16, tag="x",
                                     name="x")
                if th == 0 and mh == 0:
                    nc.sync.dma_start(xmb[:, 0:2, :],
                                      ksrc[:, 0:2, 0:1024])
                    nc.sync.dma_start(wkv[:, 2:4], wksrc[:, 2:4])
                    nc.sync.dma_start(xmb[:, 2:4, :],
                                      ksrc[:, 2:4, 0:1024])
                    nc.sync.dma_start(
                        bias_sb[:, NCT:2 * NCT],
                        bkt.rearrange("(c p) o -> p (c o)", p=128))
                    nc.sync.dma_start(wkv[:, 4:NM], wksrc[:, 4:NM])
                    nc.sync.dma_start(
                        bias_sb[:, 0:NCT],
                        bqt.rearrange("(c p) o -> p (c o)", p=128))
                    nc.sync.dma_start(ident_sb[:, :], identt)
                    nc.sync.dma_start(
                        wq_all.rearrange("p (m c) -> p m c", m=NM),
                        wqt.rearrange("(m p) c -> p m c", p=128))
                    _load_xq(0)
                else:
                    nc.sync.dma_start(
                        xmb[:, :, :],
                        ksrc[:, mh * 4:(mh + 1) * 4,
                             th * 1024:(th + 1) * 1024])
                xall[(th, mh)] = xmb
        # warm the ACT exp table while DMAs stream
        nc.gpsimd.memset(warm[:, :], 0.0)
        nc.scalar.activation(warm[:, :], warm[:, :], EXP)
        # v weights follow the k/q input stream
        nc.sync.dma_start(wv_all.rearrange("p (m c) -> p m c", m=NM),
                          wvt.rearrange("(m p) c -> p m c", p=128))
        for th in (0, 1):               # halves of T
            xh = [xall[(th, 0)], xall[(th, 1)]]
            for tc2 in (0, 1):
                npass = th * 2 + tc2
                tq = th * 2 + tc2
                for ct in range(NCT):   # sequential chains, 3 banks
                    psq = ppool.tile([128, 512], F32,
                                     tag=f"pp{(npass * 4 + ct) % 3}",
                                     name="pp")
                    for m in range(NM):
                        nc.tensor.matmul(
                            psq[:, :],
                            wk_all[:, m * C + ct * 128:
                                   m * C + (ct + 1) * 128],
                            xh[m // 4][:, m % 4,
                                       tc2 * 512:(tc2 + 1) * 512],
                            start=(m == 0), stop=(m == NM - 1))
                    nc.vector.tensor_scalar_add(
                        kpt_sb[ct][:, tq * 512:(tq + 1) * 512],
                        psq[:, :],
                        bias_sb[:, NCT + ct:NCT + ct + 1])
                    if th == 1 and tc2 == 1 and ct < 2:
                        # feed the exp stream between K's last chains
                        s1(2 + ct)
                if th == 0 and tc2 == 1:
                    # q0/hp0 here: its bias-add lands while DVE is free,
                    # so the warmup scores are not gated by K's adds.
                    emit_q_chunk(0, 0)
                if th == 1 and tc2 == 0:
                    # the exp stream starts while K's last pass projects;
                    # exactly two lead scores (= st buffers, no WAR)
                    _load_mask(0)
                    _load_vtm(0)
                    s1(0)
                    s1(1)
    ppool.release()

    # ---- Phase C: attention, with phase-D block interleaved per qc --
    otpool = tc.alloc_tile_pool(name="ot", bufs=1, space="PSUM",
                                side="right")
    trpool = tc.alloc_tile_pool(name="tr", bufs=1, space="PSUM",
                                side="right")
    with tc.tile_pool(name="nrm", bufs=2) as nrmpool, \
         tc.tile_pool(name="oq", bufs=2) as oqpool, \
         tc.tile_pool(name="yev", bufs=2) as ypool:
        NG = NQC * NCT * NKT          # 256 pipeline steps
        ots = {}                      # (qc, hp) -> [ot_h0, ot_h1]
        dstate = {}                   # rolling phase-D psum/ye tiles

        def s2(g):
            # transposed PV accumulation for step g
            qc, hp, ktile = _coords(g)
            if ktile == 0:
                ots[(qc, hp)] = [otpool.tile([128, 260], F32, tag=f"ot{h}",
                                             name=f"ot{h}")
                                 for h in range(2)]
            ot2 = ots[(qc, hp)]
            ptm = ptms.pop(g)
            for h in range(2):
                hg = hp * 2 + h
                vsl = vp_ext[:, ktile * VPW + hg * 65:
                             ktile * VPW + (hg + 1) * 65]
                for qs in range(4):
                    # one psum group per bank: the first matmul's start
                    # marks the whole zero region pending-zero, later
                    # slices replace-then-accumulate (has_written bits)
                    nc.tensor.matmul(
                        ot2[h][:, qs * 65:(qs + 1) * 65],
                        ptm[:, h * 512 + qs * 128:h * 512 + (qs + 1) * 128],
                        vsl,
                        start=(ktile == 0 and qs == 0),
                        stop=(ktile == NKT - 1 and qs == 3))

        otqs = {}                     # (qc, hp) -> [otq_h0, otq_h1]

        def epi_norm(qc, hp):
            # DVE-only: gather row sums, reciprocal, broadcast-multiply
            ot2 = ots.pop((qc, hp))
            pair = []
            for h in range(2):
                otv = ot2[h].rearrange("p (qs e) -> p qs e", qs=4)
                rsum = nrmpool.tile([128, 4, 1], F32, tag="rs", name="rsum")
                nc.vector.tensor_copy(rsum[:, :, :], otv[:, :, 64:65])
                rinv = nrmpool.tile([128, 4, 1], F32, tag="ri", name="rinv")
                nc.vector.reciprocal(rinv[:, :, :], rsum[:, :, :])
                otq = oqpool.tile([128, 256], BF16, tag=f"oq{h}",
                                  name="otq")
                nc.vector.tensor_tensor(
                    otq.rearrange("p (qs e) -> p qs e", qs=4),
                    otv[:, :, 0:64], _bcast_inner(rinv, 4, 64),
                    mybir.AluOpType.mult)
                pair.append(otq)
            otqs[(qc, hp)] = pair

        def epi_transpose(qc, hp):
            # PE transposes (identity matmul) + DVE copy psum -> otn
            pair = otqs.pop((qc, hp))
            trans = trpool.tile([128, 512], F32, tag="tr", name="trans")
            for h in range(2):
                otq = pair[h]
                for qs in range(4):
                    nc.tensor.matmul(
                        trans[h * 64:(h + 1) * 64,
                              qs * 128:(qs + 1) * 128],
                        otq[:, qs * 64:(qs + 1) * 64], ident_sb[:, :],
                        start=(qs == 0), stop=(qs == 3))
            nc.vector.tensor_copy(otn_sb[hp][:, qc * 512:(qc + 1) * 512],
                                  trans[:, :])

        LEAD = 4
        TDEFER = 3   # steps between epi_norm and epi_transpose
        # JIT projection schedules: V chunk (kt, hp) must land before
        # s2 needs vp[kt] at g = 16*hp + kt (first sweep, qc0); Q chunk
        # (tq, ct) before s1 reads qpt[ct][tq] at g = 64*tq + 16*ct.
        v_sched = {}

        def _vsched(g, kt2, vhp):
            if g % 16 == 15:     # keep epilogue steps free
                g += 1
            v_sched.setdefault(g, []).append((kt2, vhp))

        for kt2 in range(4, NKT, 2):     # pairs (kt, kt+1)
            _vsched(kt2 - 4, kt2, 0)
        for kt2 in range(0, NKT, 2):
            _vsched(kt2 + 3, kt2, 1)
            _vsched(kt2 + 21, kt2, 2)
            _vsched(kt2 + 39, kt2, 3)
        q_sched = {3: (0, 1), 18: (0, 2), 34: (0, 3)}
        xq_sched = {}
        for tq in (1, 2, 3):
            xq_sched[64 * (tq - 1) + 45] = tq
            for ct in range(NCT):
                # kt == 1 steps: clear of the D matmuls on kt 8-15
                q_sched[64 * tq + 16 * ct - 15] = (tq, ct)
        # warmup (s1(0..3) came from inside phase A)
        emit_v(0, 0)
        emit_v(2, 0)
        for g in range(NG):
            if g + LEAD < NG:
                s1(g + LEAD)
            s2(g)
            qc, hp, ktile = _coords(g)
            if g == 40:      # wo needed from the first D block (g ~ 72)
                nc.gpsimd.dma_start(
                    wo_all.rearrange("p (c j) -> p c j", c=NCT),
                    wot.rearrange("(c p) j -> p c j", p=128))
            if qc > 0 and ktile >= 8:
                # previous qcol's output projection, one matmul per step
                # (kt 8..15) so no step overruns the exp pace
                jb = hp * 2 + (ktile - 8) // 4
                ct = (ktile - 8) % 4
                if ct == 0:
                    dstate["ps"] = pypool.tile([128, 512], F32, tag="py",
                                               name="psy")
                    if ktile == 8:
                        dstate["ye"] = ypool.tile([128, 2 * 512], F32,
                                                  tag="ye", name="ye")
                nc.tensor.matmul(
                    dstate["ps"][:, :],
                    wo_all[:, ct * D + jb * 128:ct * D + (jb + 1) * 128],
                    otn_sb[ct][:, (qc - 1) * 512:qc * 512],
                    start=(ct == 0), stop=(ct == NCT - 1))
                if ct == NCT - 1:
                    nc.vector.tensor_copy(
                        dstate["ye"][:, (jb % 2) * 512:(jb % 2 + 1) * 512],
                        dstate["ps"][:, :])
                    if ktile == NKT - 1:
                        _emit_d_store(nc, dstate["ye"], yt, qc - 1, hp)
            if ktile == NKT - 1:
                epi_norm(qc, hp)
            if ktile == TDEFER - 1 and g >= NKT:
                pq, ph = _coords(g - TDEFER - (NKT - 1))[:2]
                epi_transpose(pq, ph)
            for kt2, vhp in v_sched.get(g, ()):
                emit_v(kt2, vhp)
            if g in xq_sched:
                _load_xq(xq_sched[g])
            if g in q_sched:
                emit_q_chunk(*q_sched[g])
        # Tail: final transpose, then the last qcol's 8 projection blocks
        # fully pipelined through 7 psum banks with direct psum->dram
        # stores (no intermediate sbuf copies).
        epi_transpose(NQC - 1, NCT - 1)
        trpool.release()
        otpool.release()
        stpool.release()
        dpool = tc.alloc_tile_pool(name="dtail", bufs=7, space="PSUM")
        qcl = NQC - 1
        ysink = yt.rearrange("(jb p) t -> p jb t", p=128)
        for jb in range(8):
            ps = dpool.tile([128, 512], F32, tag="d", name="psy")
            for ct in range(NCT):
                nc.tensor.matmul(
                    ps[:, :],
                    wo_all[:, ct * D + jb * 128:ct * D + (jb + 1) * 128],
                    otn_sb[ct][:, qcl * 512:(qcl + 1) * 512],
                    start=(ct == 0), stop=(ct == NCT - 1))
            ye = ypool.tile([128, 512], F32, tag=f"yd{jb % 2}", name="yed")
            if jb % 2:      # split evacuation across DVE and ACT
                nc.vector.tensor_copy(ye[:, :], ps[:, :])
            else:
                nc.scalar.activation(ye[:, :], ps[:, :],
                                     mybir.ActivationFunctionType.Copy)
            nc.sync.dma_start(
                ysink[:, jb:jb + 1, qcl * 512:(qcl + 1) * 512],
                ye.rearrange("p (o t) -> p o t", o=1))
        dpool.release()
    pypool.release()


def kernel(q, k, v, mask, Wq, bq, Wk, bk, Wv, bv, Wo, bo, _trace=False):
    if "nc" not in _CACHED:
        _CACHED["nc"] = _build_nc()
    nc = _CACHED["nc"]

    q = np.asarray(q, np.float32)
    k = np.asarray(k, np.float32)
    v = np.asarray(v, np.float32)
    Wq = np.asarray(Wq, np.float32)
    Wk = np.asarray(Wk, np.float32)
    Wv = np.asarray(Wv, np.float32)
    Wo = np.asarray(Wo, np.float32)
    mask = np.asarray(mask)
    ident = np.eye(128, dtype=np.float32).astype(ml_dtypes.bfloat16)

    in_maps = []
    for core in range(8):
        b, g = divmod(core, 2)
        csl = slice(g * C, (g + 1) * C)
        im = {
            "qt": np.ascontiguousarray(q[b].T).astype(ml_dtypes.bfloat16),
            "kt": np.ascontiguousarray(k[b].T).astype(ml_dtypes.bfloat16),
            "vt": np.ascontiguousarray(v[b].T).astype(ml_dtypes.bfloat16),
            "wqt": np.ascontiguousarray((Wq[csl, :] / 8.0).T).astype(ml_dtypes.bfloat16),
            "wkt": np.ascontiguousarray(Wk[csl, :].T).astype(ml_dtypes.bfloat16),
            "wvt": np.ascontiguousarray(Wv[csl, :].T).astype(ml_dtypes.bfloat16),
            "wot": np.ascontiguousarray(Wo[:, csl].T).astype(
                ml_dtypes.bfloat16),
            "maskt": np.ascontiguousarray(
                (~mask[b, 0]).T.astype(np.float32)).astype(ml_dtypes.bfloat16),
            "bqt": np.ascontiguousarray(
                (np.asarray(bq, np.float32)[csl] / 8.0).reshape(C, 1)),
            "bkt": np.ascontiguousarray(
                np.asarray(bk, np.float32)[csl].reshape(C, 1)),
            "identt": ident,
        }
        in_maps.append(im)

    res = bass_utils.run_bass_kernel_spmd(
        nc, in_maps, core_ids=list(range(8)), trace=_trace)
    if _trace:
        _CACHED["last_results"] = res
    outs = [r["yt"] for r in res.results]

    y = np.empty((B, T, D), np.float32)
    const = (Wo @ np.asarray(bv, np.float32)
             + np.asarray(bo, np.float32)).astype(np.float32)
    for b in range(B):
        y[b] = (outs[2 * b] + outs[2 * b + 1]).T + const
    return y


# revision 83
# speedup vs baseline: 1.2126x; 1.0003x over previous
"""Trainium2 Bass kernel for masked multi-head attention.

Reference computation (B=4, T=2048, D=1024, H=16, dh=64):
    qp = q @ Wq.T + bq ; kp = k @ Wk.T + bk ; vp = v @ Wv.T + bv
    s  = (qh @ khT) / 8 ; s = where(mask, -1e6, s) ; p = softmax(s)
    o  = p @ vh ; y = o @ Wo.T + bo

Sharding: 8 cores = (batch b in 0..3) x (head-group g in 0..1).
Each core handles batch b and 8 heads (512 channels), computes a partial
y^T (output projection over its 512 channels); host sums core pairs,
transposes, and adds the bias terms.

Per-core device algorithm (everything in transposed "T-major" layouts so
no on-device transposes are needed):
  A) qpT[c,t] = sum_m WqT[m,c] * qT[m,t]   (float32r matmuls, psum acc)
     kpT likewise.  1/8 score scale folded into WqT host-side.
  B) vp[t,c]  = sum_m vT[m,t] * WvT[m,c]   (untransposed; stored bf16 in
     a [t, 8*65] layout with a ones-column per head for row-sums)
  C) per (qcol, head): ST[k,q] = khT.T @ qhT (f32r) -> exp on ACT (bf16)
     -> multiply by maskT tile (DVE or GPSIMD, broadcast AP over both
     heads) -> transposed PV: OT[q,65] += ptm[k,q].T @ vp[k,65] per
     (head, q-subtile) with col 64 = row sums (ones column).
     Epilogue: recip rowsums (DVE), broadcast-multiply normalize (DVE),
     PE transpose via identity matmul -> [c,q] psum, DVE copy to otn.
  D) yT[j,t] = sum_c wot[c,j] * otn[c,t]   (bf16) -> DMA out.
"""

import sys
import numpy as np

for _p in ("/opt/trn_rl_repo",):
    if _p not in sys.path:
        sys.path.insert(0, _p)

import ml_dtypes
from contextlib import ExitStack

import concourse.bass as bass
import concourse.tile as tile
from concourse import bacc, mybir
from concourse import bass_utils

B, T, D, H = 4, 2048, 1024, 16
DH = 64          # head dim
HC = 8           # heads per core
C = HC * DH      # 512 channels per core
F32 = mybir.dt.float32
F32R = mybir.dt.float32r
BF16 = mybir.dt.bfloat16
EXP = mybir.ActivationFunctionType.Exp

_CACHED = {}
GP_MASK_MOD = 10 ** 9   # g % GP_MASK_MOD == 1 -> mask multiply on gpsimd


def _build_nc():
    nc = bacc.Bacc("TRN2", target_bir_lowering=False, debug=False,
                   enable_asserts=False)
    qt = nc.dram_tensor("qt", [D, T], BF16, kind="ExternalInput").ap()
    kt = nc.dram_tensor("kt", [D, T], BF16, kind="ExternalInput").ap()
    vt = nc.dram_tensor("vt", [D, T], BF16, kind="ExternalInput").ap()
    wqt = nc.dram_tensor("wqt", [D, C], BF16, kind="ExternalInput").ap()
    wkt = nc.dram_tensor("wkt", [D, C], BF16, kind="ExternalInput").ap()
    wvt = nc.dram_tensor("wvt", [D, C], BF16, kind="ExternalInput").ap()
    wot = nc.dram_tensor("wot", [C, D], BF16, kind="ExternalInput").ap()
    maskt = nc.dram_tensor("maskt", [T, T], BF16, kind="ExternalInput").ap()
    bqt = nc.dram_tensor("bqt", [C, 1], F32, kind="ExternalInput").ap()
    bkt = nc.dram_tensor("bkt", [C, 1], F32, kind="ExternalInput").ap()
    identt = nc.dram_tensor("identt", [128, 128], BF16,
                            kind="ExternalInput").ap()
    yt = nc.dram_tensor("yt", [D, T], F32, kind="ExternalOutput").ap()

    with tile.TileContext(nc) as tc, ExitStack() as ctx:
        _emit(ctx, tc, qt, kt, vt, wqt, wkt, wvt, wot, maskt, bqt, bkt,
              identt, yt)
    nc.compile()
    return nc


def _bcast(ap, reps, inner):
    """Repeat `ap`'s last `inner` elements `reps` times: [p, reps, inner]
    with a 0-stride outer dim."""
    return bass.AP(ap.tensor, ap.offset, [ap.ap[0], [0, reps], [1, inner]])


def _bcast_inner(ap, outer, reps):
    """Broadcast each of `ap`'s `outer` elements `reps` times:
    [p, outer, reps] with a 0-stride inner dim."""
    return bass.AP(ap.tensor, ap.offset, [ap.ap[0], [1, outer], [0, reps]])


def _emit_d_jbs(nc, pypool, ye, wo_all, otn_sb, qc, jbs, tag="py"):
    NCT = C // 128
    for jb in jbs:
        ps = pypool.tile([128, 512], F32, tag=tag, name="psy")
        for ct in range(NCT):
            lhs = wo_all[:, ct * D + jb * 128:ct * D + (jb + 1) * 128]
            rhs = otn_sb[ct][:, qc * 512:(qc + 1) * 512]
            nc.tensor.matmul(ps[:, :], lhs, rhs,
                             start=(ct == 0), stop=(ct == NCT - 1))
        nc.vector.tensor_copy(ye[:, (jb % 2) * 512:(jb % 2 + 1) * 512],
                              ps[:, :])


def _emit_d_store(nc, ye, yt, qc, pair):
    # gpsimd (swdge) queue: store DMAs wait on DVE ye-copies and must not
    # block input loads queued behind them on the SP queue.
    nc.gpsimd.dma_start(
        yt.rearrange("(jb p) t -> p jb t",
                     p=128)[:, pair * 2:(pair + 1) * 2,
                            qc * 512:(qc + 1) * 512],
        ye.rearrange("p (jb t) -> p jb t", jb=2))


def _emit(ctx, tc, qt, kt, vt, wqt, wkt, wvt, wot, maskt, bqt, bkt, identt,
          yt):
    nc = tc.nc
    NKT = T // 128      # 16 k-tiles
    NQC = 4             # q columns of 512
    NCT = C // 128      # 4 channel tiles (= head pairs)
    NM = D // 128       # 8 contraction tiles

    # ---- persistent SBUF arrays -------------------------------------
    persist = ctx.enter_context(tc.tile_pool(name="persist", bufs=1))
    qpt_sb = [persist.tile([128, T], BF16, tag=f"qpt{i}", name=f"qpt{i}")
              for i in range(NCT)]
    kpt_sb = [persist.tile([128, T], BF16, tag=f"kpt{i}", name=f"kpt{i}")
              for i in range(NCT)]
    VPW = HC * (DH + 1)  # 520
    vp_ext = persist.tile([128, NKT * VPW], BF16, tag="vpext", name="vpext")
    otn_sb = [persist.tile([128, T], BF16, tag=f"otn{i}", name=f"otn{i}")
              for i in range(NCT)]
    bias_sb = persist.tile([128, 2 * NCT], F32, tag="bias", name="bias")
    wo_all = persist.tile([128, NCT * D], BF16, tag="wo", name="wo_all")
    ident_sb = persist.tile([128, 128], BF16, tag="ident", name="ident_sb")
    warm = persist.tile([1, 2], F32, tag="warm", name="warm")

    # mask tiles double-buffered; qc0 mask first so attention never waits.
    mpool = ctx.enter_context(tc.tile_pool(name="mask", bufs=2))
    mask_tiles = {}
    msrc = maskt.rearrange("(kt p) q -> p kt q", p=128)

    def _load_mask(qc):
        # Later masks use the ACT dge queue: a mask load WAR-waits on DVE
        # mults of the retiring tile and must not block input loads behind
        # it on the SP queue.  qc0 (no WAR) stays on SP for strict order.
        m_all = mpool.tile([128, NKT * 512], BF16, tag="m", name="m_all")
        eng = nc.sync if qc == 0 else nc.gpsimd
        eng.dma_start(m_all.rearrange("p (kt q) -> p kt q", kt=NKT),
                      msrc[:, :, qc * 512:(qc + 1) * 512])
        mask_tiles[qc] = m_all

    nc.gpsimd.memset(vp_ext[:, :], 1.0)

    # PSUM: py(1) + st(4) right, all along; pproj(3, released after A)
    # left, then ot(2) + trans(1) right.
    pypool = tc.alloc_tile_pool(name="py", bufs=1, space="PSUM",
                                side="right")
    stpool = tc.alloc_tile_pool(name="st", bufs=2, space="PSUM",
                                side="right")
    ppool = tc.alloc_tile_pool(name="pproj", bufs=1, space="PSUM")
    ptpool = ctx.enter_context(tc.tile_pool(name="pt", bufs=4))
    ptmpool = ctx.enter_context(tc.tile_pool(name="ptm", bufs=4))
    ptms = {}                     # g -> masked-prob tile

    def _coords(g):
        qc, r = divmod(g, NCT * NKT)
        hp, ktile = divmod(r, NKT)
        return qc, hp, ktile

    def s1(g):
        # QK^T -> exp -> mask multiply for step g (LEAD ahead of PV)
        qc, hp, ktile = _coords(g)
        if hp == 1 and ktile == 0 and qc + 1 < NQC \
                and qc + 1 not in mask_tiles:
            _load_mask(qc + 1)
        m_all = mask_tiles[qc]
        st = stpool.tile([128, 1024], F32, tag="st", name="st")
        for h in range(2):
            nc.tensor.matmul(
                st[:, h * 512:(h + 1) * 512],
                kpt_sb[hp][h * 64:(h + 1) * 64,
                           ktile * 128:(ktile + 1) * 128],
                qpt_sb[hp][h * 64:(h + 1) * 64,
                           qc * 512:(qc + 1) * 512],
                start=True, stop=True)
        pt = ptpool.tile([128, 1024], BF16, tag="pt", name="pt")
        nc.scalar.activation(pt[:, :], st[:, :], EXP)
        ptm = ptmpool.tile([128, 1024], BF16, tag="ptm", name="ptm")
        msl = m_all[:, ktile * 512:(ktile + 1) * 512]
        nc.vector.tensor_tensor(
            ptm.rearrange("p (t q) -> p t q", t=2),
            pt.rearrange("p (t q) -> p t q", t=2),
            _bcast(msl, 2, 512), mybir.AluOpType.mult)
        ptms[g] = ptm

    wpoolq = ctx.enter_context(tc.tile_pool(name="wtsq", bufs=1))
    wq_all = wpoolq.tile([128, NM * C], BF16, tag="wq", name="wq_all")
    wpool2 = ctx.enter_context(tc.tile_pool(name="wts2", bufs=1))
    wv_all = wpool2.tile([128, NM * C], BF16, tag="wv", name="wv")
    vtpool = ctx.enter_context(tc.tile_pool(name="vtin", bufs=8))
    vtm_tiles = {}   # (tg, hp) -> [8 input tiles]
    xq_tiles = {}    # tq -> [8 input tiles]

    qsrc = qt.rearrange("(m p) t -> p m t", p=128)
    vsrc = vt.rearrange("(m p) t -> p m t", p=128)

    def _load_xq(tq):
        if tq in xq_tiles:
            return
        xqb = vtpool.tile([128, NM, 512], BF16, tag="xq", bufs=1,
                          name="xqb")
        nc.sync.dma_start(xqb[:, :, :],
                          qsrc[:, :, tq * 512:(tq + 1) * 512])
        xq_tiles.clear()
        xq_tiles[tq] = xqb

    def emit_q_chunk(tq, ct):
        # projects qpt[:, tq-quarter] for head-pair ct (8 matmuls, 1 bank)
        _load_xq(tq)
        xqb = xq_tiles[tq]
        ps = pypool.tile([128, 512], F32, tag="py", name="pvq")
        for m in range(NM):
            lhs = wq_all[:, m * C + ct * 128:m * C + (ct + 1) * 128]
            nc.tensor.matmul(ps[:, :], lhs, xqb[:, m, :],
                             start=(m == 0), stop=(m == NM - 1))
        nc.vector.tensor_scalar_add(
            qpt_sb[ct][:, tq * 512:(tq + 1) * 512], ps[:, :],
            bias_sb[:, ct:ct + 1])

    def _load_vtm(tg, hp=0):
        # cached per t-group (same data for every head-pair)
        if tg in vtm_tiles or not 0 <= tg < 4:
            return
        vtb = vtpool.tile([128, NM, 512], BF16, tag="vt", bufs=4,
                          name="vtb")
        nc.sync.dma_start(vtb[:, :, :],
                          vsrc[:, :, tg * 512:(tg + 1) * 512])
        vtm_tiles[tg] = vtb

    def emit_v(kt, hp):
        # V projection pair: head-pair hp, t-blocks kt and kt+1 (16
        # matmuls N=128 + one copy -> one psum-bank cycle per 2 steps)
        tg = kt // 4
        _load_vtm(tg)
        if kt % 4 == 0:     # prefetch the next t-group's inputs
            _load_vtm(tg + 1)
        vtb = vtm_tiles[tg]
        ps = pypool.tile([128, 512], F32, tag="py", name="pv")
        for j in range(2):
            i = kt % 4 + j
            for m in range(NM):
                nc.tensor.matmul(
                    ps[:, j * 128:(j + 1) * 128],
                    vtb[:, m, i * 128:(i + 1) * 128],
                    wv_all[:, m * C + hp * 128:m * C + (hp + 1) * 128],
                    start=(j == 0 and m == 0),
                    stop=(j == 1 and m == NM - 1))
        pstr = ps.ap[0][0]
        vstr = vp_ext.ap[0][0]
        dstv = bass.AP(vp_ext.tensor,
                       vp_ext.offset + kt * VPW + hp * 130,
                       [[vstr, 128], [VPW, 2], [65, 2], [1, DH]])
        srcv = bass.AP(ps.tensor, ps.offset,
                       [[pstr, 128], [128, 2], [64, 2], [1, DH]])
        nc.vector.tensor_copy(dstv, srcv)

    # ---- Phase A: K projection; q0/hp0 comes via emit_q_chunk -------
    with tc.tile_pool(name="wts", bufs=1) as wpool, \
         tc.tile_pool(name="xin", bufs=2) as xpool:
        wk_all = wpool.tile([128, NM * C], BF16, tag="w", name="wk_all")
        ksrc = kt.rearrange("(m p) t -> p m t", p=128)
        wksrc = wkt.rearrange("(m p) c -> p m c", p=128)
        wkv = wk_all.rearrange("p (m c) -> p m c", m=NM)
        # first two m-blocks of weights+inputs lead so matmuls start ~2us
        nc.sync.dma_start(wkv[:, 0:2], wksrc[:, 0:2])
        xall = {}
        for th in (0, 1):
            for mh in range(2):
                if th == 1:
                    # stage through the (still empty) vt-cache slots:
                    # later tg loads WAR-wait until pass 4 reads finish
                    xmb = vtpool.tile([128, NM // 2, 1024], BF16,
                                      tag="vt", bufs=4, name="x2")
                else:
                    xmb = xpool.tile([128, NM // 2, 1024], BF16, tag="x",
                                     name="x")
                if th == 0 and mh == 0:
                    nc.sync.dma_start(xmb[:, 0:2, :],
                                      ksrc[:, 0:2, 0:1024])
                    nc.sync.dma_start(wkv[:, 2:4], wksrc[:, 2:4])
                    nc.sync.dma_start(xmb[:, 2:4, :],
                                      ksrc[:, 2:4, 0:1024])
                    nc.sync.dma_start(
                        bias_sb[:, NCT:2 * NCT],
                        bkt.rearrange("(c p) o -> p (c o)", p=128))
                    nc.sync.dma_start(wkv[:, 4:NM], wksrc[:, 4:NM])
                    nc.sync.dma_start(
                        bias_sb[:, 0:NCT],
                        bqt.rearrange("(c p) o -> p (c o)", p=128))
                    nc.sync.dma_start(ident_sb[:, :], identt)
                    nc.sync.dma_start(
                        wq_all.rearrange("p (m c) -> p m c", m=NM),
                        wqt.rearrange("(m p) c -> p m c", p=128))
                    _load_xq(0)
                else:
                    nc.sync.dma_start(
                        xmb[:, :, :],
                        ksrc[:, mh * 4:(mh + 1) * 4,
                             th * 1024:(th + 1) * 1024])
                xall[(th, mh)] = xmb
        # warm the ACT exp table while DMAs stream
        nc.gpsimd.memset(warm[:, :], 0.0)
        nc.scalar.activation(warm[:, :], warm[:, :], EXP)
        # v weights follow the k/q input stream
        nc.sync.dma_start(wv_all.rearrange("p (m c) -> p m c", m=NM),
                          wvt.rearrange("(m p) c -> p m c", p=128))
        for th in (0, 1):               # halves of T
            xh = [xall[(th, 0)], xall[(th, 1)]]
            for tc2 in (0, 1):
                npass = th * 2 + tc2
                tq = th * 2 + tc2
                for ct in range(NCT):   # sequential chains, 3 banks
                    psq = ppool.tile([128, 512], F32,
                                     tag=f"pp{(npass * 4 + ct) % 3}",
                                     name="pp")
                    for m in range(NM):
                        nc.tensor.matmul(
                            psq[:, :],
                            wk_all[:, m * C + ct * 128:
                                   m * C + (ct + 1) * 128],
                            xh[m // 4][:, m % 4,
                                       tc2 * 512:(tc2 + 1) * 512],
                            start=(m == 0), stop=(m == NM - 1))
                    nc.vector.tensor_scalar_add(
                        kpt_sb[ct][:, tq * 512:(tq + 1) * 512],
                        psq[:, :],
                        bias_sb[:, NCT + ct:NCT + ct + 1])
                    if th == 1 and tc2 == 1 and ct < 2:
                        # feed the exp stream between K's last chains
                        s1(2 + ct)
                if th == 0 and tc2 == 1:
                    # q0/hp0 here: its bias-add lands while DVE is free,
                    # so the warmup scores are not gated by K's adds.
                    emit_q_chunk(0, 0)
                if th == 1 and tc2 == 0:
                    # the exp stream starts while K's last pass projects;
                    # exactly two lead scores (= st buffers, no WAR)
                    _load_mask(0)
                    _load_vtm(0)
                    s1(0)
                    s1(1)
    ppool.release()

    # ---- Phase C: attention, with phase-D block interleaved per qc --
    otpool = tc.alloc_tile_pool(name="ot", bufs=1, space="PSUM",
                                side="right")
    trpool = tc.alloc_tile_pool(name="tr", bufs=1, space="PSUM",
                                side="right")
    with tc.tile_pool(name="nrm", bufs=2) as nrmpool, \
         tc.tile_pool(name="oq", bufs=2) as oqpool, \
         tc.tile_pool(name="yev", bufs=2) as ypool:
        NG = NQC * NCT * NKT          # 256 pipeline steps
        ots = {}                      # (qc, hp) -> [ot_h0, ot_h1]
        dstate = {}                   # rolling phase-D psum/ye tiles

        def s2(g):
            # transposed PV accumulation for step g
            qc, hp, ktile = _coords(g)
            if ktile == 0:
                ots[(qc, hp)] = [otpool.tile([128, 260], F32, tag=f"ot{h}",
                                             name=f"ot{h}")
                                 for h in range(2)]
            ot2 = ots[(qc, hp)]
            ptm = ptms.pop(g)
            for h in range(2):
                hg = hp * 2 + h
                vsl = vp_ext[:, ktile * VPW + hg * 65:
                             ktile * VPW + (hg + 1) * 65]
                for qs in range(4):
                    # one psum group per bank: the first matmul's start
                    # marks the whole zero region pending-zero, later
                    # slices replace-then-accumulate (has_written bits)
                    nc.tensor.matmul(
                        ot2[h][:, qs * 65:(qs + 1) * 65],
                        ptm[:, h * 512 + qs * 128:h * 512 + (qs + 1) * 128],
                        vsl,
                        start=(ktile == 0 and qs == 0),
                        stop=(ktile == NKT - 1 and qs == 3))

        otqs = {}                     # (qc, hp) -> [otq_h0, otq_h1]

        def epi_norm(qc, hp):
            # DVE-only: gather row sums, reciprocal, broadcast-multiply
            ot2 = ots.pop((qc, hp))
            pair = []
            for h in range(2):
                otv = ot2[h].rearrange("p (qs e) -> p qs e", qs=4)
                rsum = nrmpool.tile([128, 4, 1], F32, tag="rs", name="rsum")
                nc.vector.tensor_copy(rsum[:, :, :], otv[:, :, 64:65])
                rinv = nrmpool.tile([128, 4, 1], F32, tag="ri", name="rinv")
                nc.vector.reciprocal(rinv[:, :, :], rsum[:, :, :])
                otq = oqpool.tile([128, 256], BF16, tag=f"oq{h}",
                                  name="otq")
                nc.vector.tensor_tensor(
                    otq.rearrange("p (qs e) -> p qs e", qs=4),
                    otv[:, :, 0:64], _bcast_inner(rinv, 4, 64),
                    mybir.AluOpType.mult)
                pair.append(otq)
            otqs[(qc, hp)] = pair

        def epi_transpose(qc, hp):
            # PE transposes (identity matmul) + DVE copy psum -> otn
            pair = otqs.pop((qc, hp))
            trans = trpool.tile([128, 512], F32, tag="tr", name="trans")
            for h in range(2):
                otq = pair[h]
                for qs in range(4):
                    nc.tensor.matmul(
                        trans[h * 64:(h + 1) * 64,
                              qs * 128:(qs + 1) * 128],
                        otq[:, qs * 64:(qs + 1) * 64], ident_sb[:, :],
                        start=(qs == 0), stop=(qs == 3))
            nc.vector.tensor_copy(otn_sb[hp][:, qc * 512:(qc + 1) * 512],
                                  trans[:, :])

        LEAD = 4
        TDEFER = 5   # steps between epi_norm and epi_transpose
        # JIT projection schedules: V chunk (kt, hp) must land before
        # s2 needs vp[kt] at g = 16*hp + kt (first sweep, qc0); Q chunk
        # (tq, ct) before s1 reads qpt[ct][tq] at g = 64*tq + 16*ct.
        v_sched = {}

        def _vsched(g, kt2, vhp):
            if g % 16 == 15:     # keep epilogue steps free
                g += 1
            v_sched.setdefault(g, []).append((kt2, vhp))

        for kt2 in range(4, NKT, 2):     # pairs (kt, kt+1)
            _vsched(kt2 - 4, kt2, 0)
        for kt2 in range(0, NKT, 2):
            _vsched(kt2 + 3, kt2, 1)
            _vsched(kt2 + 21, kt2, 2)
            _vsched(kt2 + 39, kt2, 3)
        q_sched = {3: (0, 1), 18: (0, 2), 34: (0, 3)}
        xq_sched = {}
        for tq in (1, 2, 3):
            xq_sched[64 * (tq - 1) + 45] = tq
            for ct in range(NCT):
                # kt == 1 steps: clear of the D matmuls on kt 8-15
                q_sched[64 * tq + 16 * ct - 15] = (tq, ct)
        # warmup (s1(0..3) came from inside phase A)
        emit_v(0, 0)
        emit_v(2, 0)
        for g in range(NG):
            if g + LEAD < NG:
                s1(g + LEAD)
            s2(g)
            qc, hp, ktile = _coords(g)
            if g == 40:      # wo needed from the first D block (g ~ 72)
                nc.gpsimd.dma_start(
                    wo_all.rearrange("p (c j) -> p c j", c=NCT),
                    wot.rearrange("(c p) j -> p c j", p=128))
            if qc > 0 and ktile >= 8:
                # previous qcol's output projection, one matmul per step
                # (kt 8..15) so no step overruns the exp pace
                jb = hp * 2 + (ktile - 8) // 4
                ct = (ktile - 8) % 4
                if ct == 0:
                    dstate["ps"] = pypool.tile([128, 512], F32, tag="py",
                                               name="psy")
                    if ktile == 8:
                        dstate["ye"] = ypool.tile([128, 2 * 512], F32,
                                                  tag="ye", name="ye")
                nc.tensor.matmul(
                    dstate["ps"][:, :],
                    wo_all[:, ct * D + jb * 128:ct * D + (jb + 1) * 128],
                    otn_sb[ct][:, (qc - 1) * 512:qc * 512],
                    start=(ct == 0), stop=(ct == NCT - 1))
                if ct == NCT - 1:
                    nc.vector.tensor_copy(
                        dstate["ye"][:, (jb % 2) * 512:(jb % 2 + 1) * 512],
                        dstate["ps"][:, :])
                    if ktile == NKT - 1:
                        _emit_d_store(nc, dstate["ye"], yt, qc - 1, hp)
            if ktile == NKT - 1:
                epi_norm(qc, hp)
            if ktile == TDEFER - 1 and g >= NKT:
                pq, ph = _coords(g - TDEFER - (NKT - 1))[:2]
                epi_transpose(pq, ph)
            for kt2, vhp in v_sched.get(g, ()):
                emit_v(kt2, vhp)
            if g in xq_sched:
                _load_xq(xq_sched[g])
            if g in q_sched:
                emit_q_chunk(*q_sched[g])
        # Tail: final transpose, then the last qcol's 8 projection blocks
        # fully pipelined through 7 psum banks with direct psum->dram
        # stores (no intermediate sbuf copies).
        epi_transpose(NQC - 1, NCT - 1)
        trpool.release()
        otpool.release()
        stpool.release()
        dpool = tc.alloc_tile_pool(name="dtail", bufs=7, space="PSUM")
        qcl = NQC - 1
        ysink = yt.rearrange("(jb p) t -> p jb t", p=128)
        for jb in range(8):
            ps = dpool.tile([128, 512], F32, tag="d", name="psy")
            for ct in range(NCT):
                nc.tensor.matmul(
                    ps[:, :],
                    wo_all[:, ct * D + jb * 128:ct * D + (jb + 1) * 128],
                    otn_sb[ct][:, qcl * 512:(qcl + 1) * 512],
                    start=(ct == 0), stop=(ct == NCT - 1))
            ye = ypool.tile([128, 512], F32, tag=f"yd{jb % 2}", name="yed")
            if jb % 2:      # split evacuation across DVE and ACT
                nc.vector.tensor_copy(ye[:, :], ps[:, :])
            else:
                nc.scalar.activation(ye[:, :], ps[:, :],
                                     mybir.ActivationFunctionType.Copy)
            nc.sync.dma_start(
                ysink[:, jb:jb + 1, qcl * 512:(qcl + 1) * 512],
                ye.rearrange("p (o t) -> p o t", o=1))
        dpool.release()
    pypool.release()


def kernel(q, k, v, mask, Wq, bq, Wk, bk, Wv, bv, Wo, bo, _trace=False):
    if "nc" not in _CACHED:
        _CACHED["nc"] = _build_nc()
    nc = _CACHED["nc"]

    q = np.asarray(q, np.float32)
    k = np.asarray(k, np.float32)
    v = np.asarray(v, np.float32)
    Wq = np.asarray(Wq, np.float32)
    Wk = np.asarray(Wk, np.float32)
    Wv = np.asarray(Wv, np.float32)
    Wo = np.asarray(Wo, np.float32)
    mask = np.asarray(mask)
    ident = np.eye(128, dtype=np.float32).astype(ml_dtypes.bfloat16)

    in_maps = []
    for core in range(8):
        b, g = divmod(core, 2)
        csl = slice(g * C, (g + 1) * C)
        im = {
            "qt": np.ascontiguousarray(q[b].T).astype(ml_dtypes.bfloat16),
            "kt": np.ascontiguousarray(k[b].T).astype(ml_dtypes.bfloat16),
            "vt": np.ascontiguousarray(v[b].T).astype(ml_dtypes.bfloat16),
            "wqt": np.ascontiguousarray((Wq[csl, :] / 8.0).T).astype(ml_dtypes.bfloat16),
            "wkt": np.ascontiguousarray(Wk[csl, :].T).astype(ml_dtypes.bfloat16),
            "wvt": np.ascontiguousarray(Wv[csl, :].T).astype(ml_dtypes.bfloat16),
            "wot": np.ascontiguousarray(Wo[:, csl].T).astype(
                ml_dtypes.bfloat16),
            "maskt": np.ascontiguousarray(
                (~mask[b, 0]).T.astype(np.float32)).astype(ml_dtypes.bfloat16),
            "bqt": np.ascontiguousarray(
                (np.asarray(bq, np.float32)[csl] / 8.0).reshape(C, 1)),
            "bkt": np.ascontiguousarray(
                np.asarray(bk, np.float32)[csl].reshape(C, 1)),
            "identt": ident,
        }
        in_maps.append(im)

    res = bass_utils.run_bass_kernel_spmd(
        nc, in_maps, core_ids=list(range(8)), trace=_trace)
    if _trace:
        _CACHED["last_results"] = res
    outs = [r["yt"] for r in res.results]

    y = np.empty((B, T, D), np.float32)
    const = (Wo @ np.asarray(bv, np.float32)
             + np.asarray(bo, np.float32)).astype(np.float32)
    for b in range(B):
        y[b] = (outs[2 * b] + outs[2 * b + 1]).T + const
    return y


# revision 92
# speedup vs baseline: 1.2205x; 1.0066x over previous
"""Trainium2 Bass kernel for masked multi-head attention.

Reference computation (B=4, T=2048, D=1024, H=16, dh=64):
    qp = q @ Wq.T + bq ; kp = k @ Wk.T + bk ; vp = v @ Wv.T + bv
    s  = (qh @ khT) / 8 ; s = where(mask, -1e6, s) ; p = softmax(s)
    o  = p @ vh ; y = o @ Wo.T + bo

Sharding: 8 cores = (batch b in 0..3) x (head-group g in 0..1).
Each core handles batch b and 8 heads (512 channels), computes a partial
y^T (output projection over its 512 channels); host sums core pairs,
transposes, and adds the bias terms.

Per-core device algorithm (everything in transposed "T-major" layouts so
no on-device transposes are needed):
  A) qpT[c,t] = sum_m WqT[m,c] * qT[m,t]   (float32r matmuls, psum acc)
     kpT likewise.  1/8 score scale folded into WqT host-side.
  B) vp[t,c]  = sum_m vT[m,t] * WvT[m,c]   (untransposed; stored bf16 in
     a [t, 8*65] layout with a ones-column per head for row-sums)
  C) per (qcol, head): ST[k,q] = khT.T @ qhT (f32r) -> exp on ACT (bf16)
     -> multiply by maskT tile (DVE or GPSIMD, broadcast AP over both
     heads) -> transposed PV: OT[q,65] += ptm[k,q].T @ vp[k,65] per
     (head, q-subtile) with col 64 = row sums (ones column).
     Epilogue: recip rowsums (DVE), broadcast-multiply normalize (DVE),
     PE transpose via identity matmul -> [c,q] psum, DVE copy to otn.
  D) yT[j,t] = sum_c wot[c,j] * otn[c,t]   (bf16) -> DMA out.
"""

import sys
import numpy as np

for _p in ("/opt/trn_rl_repo",):
    if _p not in sys.path:
        sys.path.insert(0, _p)

import ml_dtypes
from contextlib import ExitStack

import concourse.bass as bass
import concourse.tile as tile
from concourse import bacc, mybir
from concourse import bass_utils

B, T, D, H = 4, 2048, 1024, 16
DH = 64          # head dim
HC = 8           # heads per core
C = HC * DH      # 512 channels per core
F32 = mybir.dt.float32
F32R = mybir.dt.float32r
BF16 = mybir.dt.bfloat16
EXP = mybir.ActivationFunctionType.Exp

_CACHED = {}
GP_MASK_MOD = 10 ** 9   # g % GP_MASK_MOD == 1 -> mask multiply on gpsimd


def _build_nc():
    nc = bacc.Bacc("TRN2", target_bir_lowering=False, debug=False,
                   enable_asserts=False)
    qt = nc.dram_tensor("qt", [D, T], BF16, kind="ExternalInput").ap()
    kt = nc.dram_tensor("kt", [D, T], BF16, kind="ExternalInput").ap()
    vt = nc.dram_tensor("vt", [D, T], BF16, kind="ExternalInput").ap()
    wqt = nc.dram_tensor("wqt", [D, C], BF16, kind="ExternalInput").ap()
    wkt = nc.dram_tensor("wkt", [D, C], BF16, kind="ExternalInput").ap()
    wvt = nc.dram_tensor("wvt", [D, C], BF16, kind="ExternalInput").ap()
    wot = nc.dram_tensor("wot", [C, D], BF16, kind="ExternalInput").ap()
    maskt = nc.dram_tensor("maskt", [T, T], BF16, kind="ExternalInput").ap()
    bqt = nc.dram_tensor("bqt", [C, 1], F32, kind="ExternalInput").ap()
    bkt = nc.dram_tensor("bkt", [C, 1], F32, kind="ExternalInput").ap()
    identt = nc.dram_tensor("identt", [128, 128], BF16,
                            kind="ExternalInput").ap()
    yt = nc.dram_tensor("yt", [D, T], F32, kind="ExternalOutput").ap()

    with tile.TileContext(nc) as tc, ExitStack() as ctx:
        _emit(ctx, tc, qt, kt, vt, wqt, wkt, wvt, wot, maskt, bqt, bkt,
              identt, yt)
    nc.compile()
    return nc


def _bcast(ap, reps, inner):
    """Repeat `ap`'s last `inner` elements `reps` times: [p, reps, inner]
    with a 0-stride outer dim."""
    return bass.AP(ap.tensor, ap.offset, [ap.ap[0], [0, reps], [1, inner]])


def _bcast_inner(ap, outer, reps):
    """Broadcast each of `ap`'s `outer` elements `reps` times:
    [p, outer, reps] with a 0-stride inner dim."""
    return bass.AP(ap.tensor, ap.offset, [ap.ap[0], [1, outer], [0, reps]])


def _emit_d_jbs(nc, pypool, ye, wo_all, otn_sb, qc, jbs, tag="py"):
    NCT = C // 128
    for jb in jbs:
        ps = pypool.tile([128, 512], F32, tag=tag, name="psy")
        for ct in range(NCT):
            lhs = wo_all[:, ct * D + jb * 128:ct * D + (jb + 1) * 128]
            rhs = otn_sb[ct][:, qc * 512:(qc + 1) * 512]
            nc.tensor.matmul(ps[:, :], lhs, rhs,
                             start=(ct == 0), stop=(ct == NCT - 1))
        nc.vector.tensor_copy(ye[:, (jb % 2) * 512:(jb % 2 + 1) * 512],
                              ps[:, :])


def _emit_d_store(nc, ye, yt, qc, pair):
    # gpsimd (swdge) queue: store DMAs wait on DVE ye-copies and must not
    # block input loads queued behind them on the SP queue.
    nc.gpsimd.dma_start(
        yt.rearrange("(jb p) t -> p jb t",
                     p=128)[:, pair * 2:(pair + 1) * 2,
                            qc * 512:(qc + 1) * 512],
        ye.rearrange("p (jb t) -> p jb t", jb=2))


def _emit(ctx, tc, qt, kt, vt, wqt, wkt, wvt, wot, maskt, bqt, bkt, identt,
          yt):
    nc = tc.nc
    NKT = T // 128      # 16 k-tiles
    NQC = 4             # q columns of 512
    NCT = C // 128      # 4 channel tiles (= head pairs)
    NM = D // 128       # 8 contraction tiles

    # ---- persistent SBUF arrays -------------------------------------
    persist = ctx.enter_context(tc.tile_pool(name="persist", bufs=1))
    qpt_sb = [persist.tile([128, T], BF16, tag=f"qpt{i}", name=f"qpt{i}")
              for i in range(NCT)]
    kpt_sb = [persist.tile([128, T], BF16, tag=f"kpt{i}", name=f"kpt{i}")
              for i in range(NCT)]
    VPW = HC * (DH + 1)  # 520
    vp_ext = persist.tile([128, NKT * VPW], BF16, tag="vpext", name="vpext")
    otn_sb = [persist.tile([128, T], BF16, tag=f"otn{i}", name=f"otn{i}")
              for i in range(NCT)]
    bias_sb = persist.tile([128, 2 * NCT], F32, tag="bias", name="bias")
    wo_all = persist.tile([128, NCT * D], BF16, tag="wo", name="wo_all")
    ident_sb = persist.tile([128, 128], BF16, tag="ident", name="ident_sb")
    warm = persist.tile([1, 2], F32, tag="warm", name="warm")

    # mask tiles double-buffered; qc0 mask first so attention never waits.
    mpool = ctx.enter_context(tc.tile_pool(name="mask", bufs=2))
    mask_tiles = {}
    msrc = maskt.rearrange("(kt p) q -> p kt q", p=128)

    def _load_mask(qc):
        # Later masks use the ACT dge queue: a mask load WAR-waits on DVE
        # mults of the retiring tile and must not block input loads behind
        # it on the SP queue.  qc0 (no WAR) stays on SP for strict order.
        m_all = mpool.tile([128, NKT * 512], BF16, tag="m", name="m_all")
        eng = nc.sync if qc == 0 else nc.gpsimd
        eng.dma_start(m_all.rearrange("p (kt q) -> p kt q", kt=NKT),
                      msrc[:, :, qc * 512:(qc + 1) * 512])
        mask_tiles[qc] = m_all

    nc.gpsimd.memset(vp_ext[:, :], 1.0)

    # PSUM: py(1) + st(4) right, all along; pproj(3, released after A)
    # left, then ot(2) + trans(1) right.
    pypool = tc.alloc_tile_pool(name="py", bufs=1, space="PSUM",
                                side="right")
    stpool = tc.alloc_tile_pool(name="st", bufs=2, space="PSUM",
                                side="right")
    ppool = tc.alloc_tile_pool(name="pproj", bufs=1, space="PSUM")
    ptpool = ctx.enter_context(tc.tile_pool(name="pt", bufs=4))
    ptmpool = ctx.enter_context(tc.tile_pool(name="ptm", bufs=7))
    ptms = {}                     # g -> masked-prob tile

    def _coords(g):
        qc, r = divmod(g, NCT * NKT)
        hp, ktile = divmod(r, NKT)
        return qc, hp, ktile

    def s1(g):
        # QK^T -> exp -> mask multiply for step g (LEAD ahead of PV)
        qc, hp, ktile = _coords(g)
        if hp == 1 and ktile == 0 and qc + 1 < NQC \
                and qc + 1 not in mask_tiles:
            _load_mask(qc + 1)
        m_all = mask_tiles[qc]
        st = stpool.tile([128, 1024], F32, tag="st", name="st")
        for h in range(2):
            nc.tensor.matmul(
                st[:, h * 512:(h + 1) * 512],
                kpt_sb[hp][h * 64:(h + 1) * 64,
                           ktile * 128:(ktile + 1) * 128],
                qpt_sb[hp][h * 64:(h + 1) * 64,
                           qc * 512:(qc + 1) * 512],
                start=True, stop=True)
        pt = ptpool.tile([128, 1024], BF16, tag="pt", name="pt")
        nc.scalar.activation(pt[:, :], st[:, :], EXP)
        ptm = ptmpool.tile([128, 1024], BF16, tag="ptm", name="ptm")
        msl = m_all[:, ktile * 512:(ktile + 1) * 512]
        nc.vector.tensor_tensor(
            ptm.rearrange("p (t q) -> p t q", t=2),
            pt.rearrange("p (t q) -> p t q", t=2),
            _bcast(msl, 2, 512), mybir.AluOpType.mult)
        ptms[g] = ptm

    wpoolq = ctx.enter_context(tc.tile_pool(name="wtsq", bufs=1))
    wq_all = wpoolq.tile([128, NM * C], BF16, tag="wq", name="wq_all")
    wpool2 = ctx.enter_context(tc.tile_pool(name="wts2", bufs=1))
    wv_all = wpool2.tile([128, NM * C], BF16, tag="wv", name="wv")
    vtpool = ctx.enter_context(tc.tile_pool(name="vtin", bufs=8))
    vtm_tiles = {}   # (tg, hp) -> [8 input tiles]
    xq_tiles = {}    # tq -> [8 input tiles]

    qsrc = qt.rearrange("(m p) t -> p m t", p=128)
    vsrc = vt.rearrange("(m p) t -> p m t", p=128)

    def _load_xq(tq):
        if tq in xq_tiles:
            return
        xqb = vtpool.tile([128, NM, 512], BF16, tag="xq", bufs=1,
                          name="xqb")
        nc.sync.dma_start(xqb[:, :, :],
                          qsrc[:, :, tq * 512:(tq + 1) * 512])
        xq_tiles.clear()
        xq_tiles[tq] = xqb

    def emit_q_chunk(tq, ct):
        # projects qpt[:, tq-quarter] for head-pair ct (8 matmuls, 1 bank)
        _load_xq(tq)
        xqb = xq_tiles[tq]
        ps = pypool.tile([128, 512], F32, tag="py", name="pvq")
        for m in range(NM):
            lhs = wq_all[:, m * C + ct * 128:m * C + (ct + 1) * 128]
            nc.tensor.matmul(ps[:, :], lhs, xqb[:, m, :],
                             start=(m == 0), stop=(m == NM - 1))
        nc.vector.tensor_scalar_add(
            qpt_sb[ct][:, tq * 512:(tq + 1) * 512], ps[:, :],
            bias_sb[:, ct:ct + 1])

    def _load_vtm(tg, hp=0):
        # cached per t-group (same data for every head-pair)
        if tg in vtm_tiles or not 0 <= tg < 4:
            return
        vtb = vtpool.tile([128, NM, 512], BF16, tag="vt", bufs=4,
                          name="vtb")
        nc.sync.dma_start(vtb[:, :, :],
                          vsrc[:, :, tg * 512:(tg + 1) * 512])
        vtm_tiles[tg] = vtb

    def emit_v(kt, hp):
        # V projection pair: head-pair hp, t-blocks kt and kt+1 (16
        # matmuls N=128 + one copy -> one psum-bank cycle per 2 steps)
        tg = kt // 4
        _load_vtm(tg)
        if kt % 4 == 0:     # prefetch the next t-group's inputs
            _load_vtm(tg + 1)
        vtb = vtm_tiles[tg]
        ps = pypool.tile([128, 512], F32, tag="py", name="pv")
        for j in range(2):
            i = kt % 4 + j
            for m in range(NM):
                nc.tensor.matmul(
                    ps[:, j * 128:(j + 1) * 128],
                    vtb[:, m, i * 128:(i + 1) * 128],
                    wv_all[:, m * C + hp * 128:m * C + (hp + 1) * 128],
                    start=(j == 0 and m == 0),
                    stop=(j == 1 and m == NM - 1))
        pstr = ps.ap[0][0]
        vstr = vp_ext.ap[0][0]
        dstv = bass.AP(vp_ext.tensor,
                       vp_ext.offset + kt * VPW + hp * 130,
                       [[vstr, 128], [VPW, 2], [65, 2], [1, DH]])
        srcv = bass.AP(ps.tensor, ps.offset,
                       [[pstr, 128], [128, 2], [64, 2], [1, DH]])
        nc.vector.tensor_copy(dstv, srcv)

    # ---- Phase A: K projection; q0/hp0 comes via emit_q_chunk -------
    with tc.tile_pool(name="wts", bufs=1) as wpool, \
         tc.tile_pool(name="xin", bufs=2) as xpool:
        wk_all = wpool.tile([128, NM * C], BF16, tag="w", name="wk_all")
        ksrc = kt.rearrange("(m p) t -> p m t", p=128)
        wksrc = wkt.rearrange("(m p) c -> p m c", p=128)
        wkv = wk_all.rearrange("p (m c) -> p m c", m=NM)
        # first two m-blocks of weights+inputs lead so matmuls start ~2us
        nc.sync.dma_start(wkv[:, 0:2], wksrc[:, 0:2])
        xall = {}
        for th in (0, 1):
            for mh in range(2):
                if th == 1:
                    # stage through the (still empty) vt-cache slots:
                    # later tg loads WAR-wait until pass 4 reads finish
                    xmb = vtpool.tile([128, NM // 2, 1024], BF16,
                                      tag="vt", bufs=4, name="x2")
                else:
                    xmb = xpool.tile([128, NM // 2, 1024], BF16, tag="x",
                                     name="x")
                if th == 0 and mh == 0:
                    nc.sync.dma_start(xmb[:, 0:2, :],
                                      ksrc[:, 0:2, 0:1024])
                    nc.sync.dma_start(wkv[:, 2:4], wksrc[:, 2:4])
                    nc.sync.dma_start(xmb[:, 2:4, :],
                                      ksrc[:, 2:4, 0:1024])
                    nc.sync.dma_start(
                        bias_sb[:, NCT:2 * NCT],
                        bkt.rearrange("(c p) o -> p (c o)", p=128))
                    nc.sync.dma_start(wkv[:, 4:NM], wksrc[:, 4:NM])
                    nc.sync.dma_start(
                        bias_sb[:, 0:NCT],
                        bqt.rearrange("(c p) o -> p (c o)", p=128))
                    nc.sync.dma_start(ident_sb[:, :], identt)
                    nc.sync.dma_start(
                        wq_all.rearrange("p (m c) -> p m c", m=NM),
                        wqt.rearrange("(m p) c -> p m c", p=128))
                    _load_xq(0)
                else:
                    nc.sync.dma_start(
                        xmb[:, :, :],
                        ksrc[:, mh * 4:(mh + 1) * 4,
                             th * 1024:(th + 1) * 1024])
                xall[(th, mh)] = xmb
        # warm the ACT exp table while DMAs stream
        nc.gpsimd.memset(warm[:, :], 0.0)
        nc.scalar.activation(warm[:, :], warm[:, :], EXP)
        # v weights follow the k/q input stream
        nc.sync.dma_start(wv_all.rearrange("p (m c) -> p m c", m=NM),
                          wvt.rearrange("(m p) c -> p m c", p=128))
        for th in (0, 1):               # halves of T
            xh = [xall[(th, 0)], xall[(th, 1)]]
            for tc2 in (0, 1):
                npass = th * 2 + tc2
                tq = th * 2 + tc2
                for ct in range(NCT):   # sequential chains, 3 banks
                    psq = ppool.tile([128, 512], F32,
                                     tag=f"pp{(npass * 4 + ct) % 3}",
                                     name="pp")
                    for m in range(NM):
                        nc.tensor.matmul(
                            psq[:, :],
                            wk_all[:, m * C + ct * 128:
                                   m * C + (ct + 1) * 128],
                            xh[m // 4][:, m % 4,
                                       tc2 * 512:(tc2 + 1) * 512],
                            start=(m == 0), stop=(m == NM - 1))
                    nc.vector.tensor_scalar_add(
                        kpt_sb[ct][:, tq * 512:(tq + 1) * 512],
                        psq[:, :],
                        bias_sb[:, NCT + ct:NCT + ct + 1])
                    if th == 1 and tc2 == 1:
                        # feed the exp stream between K's last chains
                        s1(2 + ct)
                if th == 0 and tc2 == 1:
                    # q0/hp0 here: its bias-add lands while DVE is free,
                    # so the warmup scores are not gated by K's adds.
                    emit_q_chunk(0, 0)
                if th == 1 and tc2 == 0:
                    # the exp stream starts while K's last pass projects;
                    # exactly two lead scores (= st buffers, no WAR)
                    _load_mask(0)
                    _load_vtm(0)
                    s1(0)
                    s1(1)
    ppool.release()

    # ---- Phase C: attention, with phase-D block interleaved per qc --
    otpool = tc.alloc_tile_pool(name="ot", bufs=1, space="PSUM",
                                side="right")
    trpool = tc.alloc_tile_pool(name="tr", bufs=1, space="PSUM",
                                side="right")
    with tc.tile_pool(name="nrm", bufs=2) as nrmpool, \
         tc.tile_pool(name="oq", bufs=2) as oqpool, \
         tc.tile_pool(name="yev", bufs=2) as ypool:
        NG = NQC * NCT * NKT          # 256 pipeline steps
        ots = {}                      # (qc, hp) -> [ot_h0, ot_h1]
        dstate = {}                   # rolling phase-D psum/ye tiles

        def s2(g):
            # transposed PV accumulation for step g
            qc, hp, ktile = _coords(g)
            if ktile == 0:
                ots[(qc, hp)] = [otpool.tile([128, 260], F32, tag=f"ot{h}",
                                             name=f"ot{h}")
                                 for h in range(2)]
            ot2 = ots[(qc, hp)]
            ptm = ptms.pop(g)
            for h in range(2):
                hg = hp * 2 + h
                vsl = vp_ext[:, ktile * VPW + hg * 65:
                             ktile * VPW + (hg + 1) * 65]
                for qs in range(4):
                    # one psum group per bank: the first matmul's start
                    # marks the whole zero region pending-zero, later
                    # slices replace-then-accumulate (has_written bits)
                    nc.tensor.matmul(
                        ot2[h][:, qs * 65:(qs + 1) * 65],
                        ptm[:, h * 512 + qs * 128:h * 512 + (qs + 1) * 128],
                        vsl,
                        start=(ktile == 0 and qs == 0),
                        stop=(ktile == NKT - 1 and qs == 3))

        otqs = {}                     # (qc, hp) -> [otq_h0, otq_h1]

        def epi_norm(qc, hp):
            # DVE-only: gather row sums, reciprocal, broadcast-multiply
            ot2 = ots.pop((qc, hp))
            pair = []
            for h in range(2):
                otv = ot2[h].rearrange("p (qs e) -> p qs e", qs=4)
                rinv = nrmpool.tile([128, 4, 1], F32, tag="ri", name="rinv")
                nc.vector.reciprocal(rinv[:, :, :], otv[:, :, 64:65])
                otq = oqpool.tile([128, 256], BF16, tag=f"oq{h}",
                                  name="otq")
                nc.vector.tensor_tensor(
                    otq.rearrange("p (qs e) -> p qs e", qs=4),
                    otv[:, :, 0:64], _bcast_inner(rinv, 4, 64),
                    mybir.AluOpType.mult)
                pair.append(otq)
            otqs[(qc, hp)] = pair

        def epi_transpose(qc, hp):
            # PE transposes (identity matmul) + DVE copy psum -> otn
            pair = otqs.pop((qc, hp))
            trans = trpool.tile([128, 512], F32, tag="tr", name="trans")
            for h in range(2):
                otq = pair[h]
                for qs in range(4):
                    nc.tensor.matmul(
                        trans[h * 64:(h + 1) * 64,
                              qs * 128:(qs + 1) * 128],
                        otq[:, qs * 64:(qs + 1) * 64], ident_sb[:, :],
                        start=(qs == 0), stop=(qs == 3))
            nc.vector.tensor_copy(otn_sb[hp][:, qc * 512:(qc + 1) * 512],
                                  trans[:, :])

        LEAD = 6
        TDEFER = 5   # steps between epi_norm and epi_transpose
        # JIT projection schedules: V chunk (kt, hp) must land before
        # s2 needs vp[kt] at g = 16*hp + kt (first sweep, qc0); Q chunk
        # (tq, ct) before s1 reads qpt[ct][tq] at g = 64*tq + 16*ct.
        v_sched = {}

        def _vsched(g, kt2, vhp):
            if g % 16 == 15:     # keep epilogue steps free
                g += 1
            v_sched.setdefault(g, []).append((kt2, vhp))

        for kt2 in range(4, NKT, 2):     # pairs (kt, kt+1)
            _vsched(kt2 - 4, kt2, 0)
        for kt2 in range(0, NKT, 2):
            _vsched(kt2 + 3, kt2, 1)
            _vsched(kt2 + 21, kt2, 2)
            _vsched(kt2 + 39, kt2, 3)
        q_sched = {3: (0, 1), 18: (0, 2), 34: (0, 3)}
        xq_sched = {}
        for tq in (1, 2, 3):
            xq_sched[64 * (tq - 1) + 45] = tq
            for ct in range(NCT):
                # kt == 1 steps: clear of the D matmuls on kt 8-15
                q_sched[64 * tq + 16 * ct - 15] = (tq, ct)
        # warmup (s1(0..3) came from inside phase A)
        emit_v(0, 0)
        emit_v(2, 0)
        for g in range(NG):
            if 5 < g + LEAD < NG:   # s1(0..5) pre-emitted
                s1(g + LEAD)
            s2(g)
            qc, hp, ktile = _coords(g)
            if g == 40:      # wo needed from the first D block (g ~ 72)
                nc.gpsimd.dma_start(
                    wo_all.rearrange("p (c j) -> p c j", c=NCT),
                    wot.rearrange("(c p) j -> p c j", p=128))
            if qc > 0 and ktile >= 8:
                # previous qcol's output projection, one matmul per step
                # (kt 8..15) so no step overruns the exp pace
                jb = hp * 2 + (ktile - 8) // 4
                ct = (ktile - 8) % 4
                if ct == 0:
                    dstate["ps"] = pypool.tile([128, 512], F32, tag="py",
                                               name="psy")
                    if ktile == 8:
                        dstate["ye"] = ypool.tile([128, 2 * 512], F32,
                                                  tag="ye", name="ye")
                nc.tensor.matmul(
                    dstate["ps"][:, :],
                    wo_all[:, ct * D + jb * 128:ct * D + (jb + 1) * 128],
                    otn_sb[ct][:, (qc - 1) * 512:qc * 512],
                    start=(ct == 0), stop=(ct == NCT - 1))
                if ct == NCT - 1:
                    nc.vector.tensor_copy(
                        dstate["ye"][:, (jb % 2) * 512:(jb % 2 + 1) * 512],
                        dstate["ps"][:, :])
                    if ktile == NKT - 1:
                        _emit_d_store(nc, dstate["ye"], yt, qc - 1, hp)
            if ktile == NKT - 1:
                epi_norm(qc, hp)
            if ktile == TDEFER - 1 and g >= NKT:
                pq, ph = _coords(g - TDEFER - (NKT - 1))[:2]
                epi_transpose(pq, ph)
            for kt2, vhp in v_sched.get(g, ()):
                emit_v(kt2, vhp)
            if g in xq_sched:
                _load_xq(xq_sched[g])
            if g in q_sched:
                emit_q_chunk(*q_sched[g])
        # Tail: final transpose, then the last qcol's 8 projection blocks
        # fully pipelined through 7 psum banks with direct psum->dram
        # stores (no intermediate sbuf copies).
        epi_transpose(NQC - 1, NCT - 1)
        trpool.release()
        otpool.release()
        stpool.release()
        dpool = tc.alloc_tile_pool(name="dtail", bufs=7, space="PSUM")
        qcl = NQC - 1
        ysink = yt.rearrange("(jb p) t -> p jb t", p=128)
        for jb in range(8):
            ps = dpool.tile([128, 512], F32, tag="d", name="psy")
            for ct in range(NCT):
                nc.tensor.matmul(
                    ps[:, :],
                    wo_all[:, ct * D + jb * 128:ct * D + (jb + 1) * 128],
                    otn_sb[ct][:, qcl * 512:(qcl + 1) * 512],
                    start=(ct == 0), stop=(ct == NCT - 1))
            ye = ypool.tile([128, 512], F32, tag=f"yd{jb % 2}", name="yed")
            if jb % 2:      # split evacuation across DVE and ACT
                nc.vector.tensor_copy(ye[:, :], ps[:, :])
            else:
                nc.scalar.activation(ye[:, :], ps[:, :],
                                     mybir.ActivationFunctionType.Copy)
            nc.sync.dma_start(
                ysink[:, jb:jb + 1, qcl * 512:(qcl + 1) * 512],
                ye.rearrange("p (o t) -> p o t", o=1))
        dpool.release()
    pypool.release()


def kernel(q, k, v, mask, Wq, bq, Wk, bk, Wv, bv, Wo, bo, _trace=False):
    if "nc" not in _CACHED:
        _CACHED["nc"] = _build_nc()
    nc = _CACHED["nc"]

    q = np.asarray(q, np.float32)
    k = np.asarray(k, np.float32)
    v = np.asarray(v, np.float32)
    Wq = np.asarray(Wq, np.float32)
    Wk = np.asarray(Wk, np.float32)
    Wv = np.asarray(Wv, np.float32)
    Wo = np.asarray(Wo, np.float32)
    mask = np.asarray(mask)
    ident = np.eye(128, dtype=np.float32).astype(ml_dtypes.bfloat16)

    in_maps = []
    for core in range(8):
        b, g = divmod(core, 2)
        csl = slice(g * C, (g + 1) * C)
        im = {
            "qt": np.ascontiguousarray(q[b].T).astype(ml_dtypes.bfloat16),
            "kt": np.ascontiguousarray(k[b].T).astype(ml_dtypes.bfloat16),
            "vt": np.ascontiguousarray(v[b].T).astype(ml_dtypes.bfloat16),
            "wqt": np.ascontiguousarray((Wq[csl, :] / 8.0).T).astype(ml_dtypes.bfloat16),
            "wkt": np.ascontiguousarray(Wk[csl, :].T).astype(ml_dtypes.bfloat16),
            "wvt": np.ascontiguousarray(Wv[csl, :].T).astype(ml_dtypes.bfloat16),
            "wot": np.ascontiguousarray(Wo[:, csl].T).astype(
                ml_dtypes.bfloat16),
            "maskt": np.ascontiguousarray(
                (~mask[b, 0]).T.astype(np.float32)).astype(ml_dtypes.bfloat16),
            "bqt": np.ascontiguousarray(
                (np.asarray(bq, np.float32)[csl] / 8.0).reshape(C, 1)),
            "bkt": np.ascontiguousarray(
                np.asarray(bk, np.float32)[csl].reshape(C, 1)),
            "identt": ident,
        }
        in_maps.append(im)

    res = bass_utils.run_bass_kernel_spmd(
        nc, in_maps, core_ids=list(range(8)), trace=_trace)
    if _trace:
        _CACHED["last_results"] = res
    outs = [r["yt"] for r in res.results]

    y = np.empty((B, T, D), np.float32)
    const = (Wo @ np.asarray(bv, np.float32)
             + np.asarray(bo, np.float32)).astype(np.float32)
    for b in range(B):
        y[b] = (outs[2 * b] + outs[2 * b + 1]).T + const
    return y
